# revision 2
# baseline (speedup 1.0000x reference)
"""Trainium2 Bass kernel for a dense-transformer attention block.

Module: y = o_proj(causal_sdpa(rope(q_proj(x)), rope(k_proj(x)), v_proj(x)))
Shapes: x [2, 2048, 2048], 32 q heads / 8 kv heads, head_dim 64, fp32 I/O.

Sharding (8 NeuronCores): 2-way data parallel over batch x 4-way tensor
parallel over heads. Core c handles batch c//4 and head group c%4
(8 q heads, 2 kv heads). Each core produces a partial [2048, 2048]
output (its heads' slice of o_proj); the host sums the 4 partials per
batch.

v2 design notes (evidence from the v1 NTFF profile):
- The qkv projection phase and the attention phase are INTERLEAVED at
  emission: attention for query-chunk qc only needs q/k/v tiles
  0..4qc+3, so qkv for t-block b+1 is emitted between the heads of
  attention block b. This overlaps the ACT engine's exp work (~197us
  total, the near-critical resource) with qkv PE work instead of
  cramming it into a trailing phase.
- All q/k/o transposes go through the DMA XBAR (dma_start_transpose,
  16x128 tiles) instead of PE transpose + DVE eviction: q is transposed
  in 2-head-stacked [128,128] blocks; k is staged in half-zeroed
  [128,128] blocks so the transposed outputs are the zero-padded
  kT-lo/kT-hi stationaries the S matmuls need.
- Input DMA: x arrives as 4 per-t-block tensors (tile-granular deps,
  ~5.6us quantum) on the sync HWDGE queue; weights per-k on the scalar
  HWDGE queue. Output DMA goes through the gpsimd SWDGE queue to keep
  both HWDGE queues free for transposes/loads.
- RoPE is 3 DVE ops per projection (signed-sin table + swapped-half
  access pattern); softmax normalization is 1 reciprocal + 1
  broadcast-multiply per (qc, h) writing all 4 query tiles at once.
- PSUM: psB(qkv accum, 1 buf)=2 banks, psS(S^T fp32, 2 bufs)=4,
  psO(O accum, 1 buf)=1, psF(o_proj, 1 buf)=1 -> exactly 8 banks.
  Scores stay fp32 (TRN2 matmul cannot write 16-bit PSUM).
- Scores are computed transposed (ST = k q^T per 128x(<=512) tile), exp
  on eviction (no max subtraction: |0.125*S| < ~10 here), exp(ST) is
  the stationary P^T of the O matmul; a ones-column appended to V
  accumulates the softmax denominator as O column 64.
"""

import os
import sys
import types

import numpy as np

sys.path.insert(0, "/opt/trn_rl_repo")

import concourse.bacc as bacc  # noqa: E402
import concourse.bass as bass  # noqa: E402
import concourse.tile as tile  # noqa: E402
from concourse import mybir  # noqa: E402
from concourse.bass_utils import run_bass_kernel_spmd  # noqa: E402

try:
    import ml_dtypes
    BF16 = ml_dtypes.bfloat16
except ImportError:  # pragma: no cover
    BF16 = np.dtype("bfloat16")

HIDDEN = 2048
SEQ = 2048
BATCH = 2
N_HEADS = 32
N_KV_HEADS = 8
HEAD_DIM = 64
ROPE_THETA = 10000.0

N_CORES = 8
TP = 4                      # head-parallel ways
QH = N_HEADS // TP          # 8 q heads per core
KVH = N_KV_HEADS // TP      # 2 kv heads per core
KT = HIDDEN // 128          # 16 contraction tiles
TT = SEQ // 128             # 16 seq tiles
F_QKV = QH * HEAD_DIM + 2 * KVH * HEAD_DIM  # 512 + 128 + 128 = 768
F_O = QH * HEAD_DIM         # 512
HD = HEAD_DIM

FP32 = mybir.dt.float32
BF16_DT = mybir.dt.bfloat16


def _build_nc():
    nc = bacc.Bacc("TRN2", target_bir_lowering=False, debug=False)

    xT = nc.dram_tensor("xT", [HIDDEN, SEQ], BF16_DT, kind="ExternalInput")
    wqkv = nc.dram_tensor("wqkv", [HIDDEN, F_QKV], BF16_DT, kind="ExternalInput")
    wo = nc.dram_tensor("wo", [F_O, HIDDEN], BF16_DT, kind="ExternalInput")
    cos = nc.dram_tensor("cos", [SEQ, HEAD_DIM], FP32, kind="ExternalInput")
    ss = nc.dram_tensor("ss", [SEQ, HEAD_DIM], FP32, kind="ExternalInput")
    maskt = nc.dram_tensor("maskt", [128, 128], BF16_DT, kind="ExternalInput")
    out = nc.dram_tensor("out", [SEQ, HIDDEN], FP32, kind="ExternalOutput")

    with tile.TileContext(nc) as tc:
        _emit(nc, tc, xT, wqkv, wo, cos, ss, maskt, out)
    nc.compile()
    return nc


def _bcast(ap, n, axis_pos=1):
    """Insert a step-0 (broadcast) dim of size n into an AP at axis_pos."""
    new = list(ap.ap)
    new.insert(axis_pos, [0, n])
    return bass.AP(tensor=ap.tensor, offset=ap.offset, ap=new)


def _bcast_last(ap, n):
    """Append a step-0 (broadcast) dim of size n to an AP."""
    return bass.AP(tensor=ap.tensor, offset=ap.offset, ap=list(ap.ap) + [[0, n]])


def _swap_half(ap3):
    """[128, H, 64] view -> [128, H, 2, 32] enumerating cols (32:64, 0:32).

    Used for RoPE rotate-half: element (h, b, d) reads src col
    (32..63, 0..31)[b*32+d] of head h.
    """
    hi = ap3[:, :, 32:64]
    new = list(hi.ap)
    new.insert(len(new) - 1, [-32, 2])
    return bass.AP(tensor=hi.tensor, offset=hi.offset, ap=new)


def _ss_view(ss_t, h):
    """[128, 64] signed-sin slice -> broadcast [128, H, 2, 32] view."""
    a = list(ss_t.ap)
    return bass.AP(tensor=ss_t.tensor, offset=ss_t.offset,
                   ap=[a[0], [0, h], [32, 2], [1, 32]])


def _emit(nc, tc, xT, wqkv, wo, cos, ss, maskt, out):
    from contextlib import ExitStack
    ctx = ExitStack()
    Exp = mybir.ActivationFunctionType.Exp
    mult = mybir.AluOpType.mult

    const = ctx.enter_context(tc.tile_pool(name="const", bufs=1))
    persist = ctx.enter_context(tc.tile_pool(name="persist", bufs=1))

    # ---- input DMA: weights/tables on the scalar HWDGE queue ----
    wk = [const.tile([128, F_QKV], BF16_DT, name=f"wk{k}") for k in range(KT)]
    w_r = wqkv[:].rearrange("(k p) f -> p k f", p=128)
    for k in range(KT):
        nc.scalar.dma_start(out=wk[k][:], in_=w_r[:, k, :])
    cos_sb = const.tile([128, TT, HD], FP32)
    ss_sb = const.tile([128, TT, HD], FP32)
    nc.scalar.dma_start(out=cos_sb[:], in_=cos[:].rearrange("(t p) d -> p t d", p=128))
    nc.scalar.dma_start(out=ss_sb[:], in_=ss[:].rearrange("(t p) d -> p t d", p=128))
    mask_sb = const.tile([128, 128], BF16_DT)
    nc.scalar.dma_start(out=mask_sb[:], in_=maskt[:])
    wo_sb = const.tile([128, F_O // 128, HIDDEN], BF16_DT)
    nc.scalar.dma_start(out=wo_sb[:], in_=wo[:].rearrange("(k p) d -> p k d", p=128))

    # ---- input DMA: x as 4 per-t-block tensors on the sync HWDGE queue ----
    xb = [const.tile([128, KT, 512], BF16_DT, name=f"xb{b}") for b in range(4)]
    xT_r = xT[:].rearrange("(k p) t -> p k t", p=128)
    for b in range(4):
        nc.sync.dma_start(out=xb[b][:], in_=xT_r[:, :, bass.ts(b, 512)])

    # ---- persistent SBUF tensors ----
    qT = [persist.tile([128, SEQ], BF16_DT, name=f"qT{p}") for p in range(4)]
    kTlo = persist.tile([128, KVH, SEQ], BF16_DT, name="kTlo")
    kThi = persist.tile([128, KVH, SEQ], BF16_DT, name="kThi")
    v_sb = [persist.tile([128, KVH, HD + 1], BF16_DT, name=f"v{t}")
            for t in range(TT)]
    o_sb = [persist.tile([128, 4, F_O], BF16_DT, name=f"ob{qc}") for qc in range(4)]
    oT_sb = [persist.tile([128, 512], BF16_DT, name=f"oT{f}") for f in range(4)]
    # k XBAR staging: [k_rot | zeros] and [zeros | k_rot]; zero halves are
    # memset once and never rewritten. Two buffers, alternating by t.
    k2lo = [persist.tile([128, KVH, 128], BF16_DT, name=f"k2lo{i}") for i in range(2)]
    k2hi = [persist.tile([128, KVH, 128], BF16_DT, name=f"k2hi{i}") for i in range(2)]
    for i in range(2):
        nc.gpsimd.memset(k2lo[i][:, :, HD:128], 0.0)
        nc.gpsimd.memset(k2hi[i][:, :, 0:HD], 0.0)
    for t in range(TT):
        nc.gpsimd.memset(v_sb[t][:, :, HD:HD + 1], 1.0)

    # ---- pools ----
    psB = ctx.enter_context(tc.tile_pool(name="psB", bufs=1, space="PSUM"))
    psS = ctx.enter_context(tc.tile_pool(name="psS", bufs=2, space="PSUM"))
    psO = ctx.enter_context(tc.tile_pool(name="psO", bufs=1, space="PSUM"))
    psF = ctx.enter_context(tc.tile_pool(name="psF", bufs=1, space="PSUM"))
    bwork = ctx.enter_context(tc.tile_pool(name="bwork", bufs=2))
    att = ctx.enter_context(tc.tile_pool(name="att", bufs=8))
    fwork = ctx.enter_context(tc.tile_pool(name="fwork", bufs=3))

    def rope3(dst3, src3, nheads, cos_t, ss_t, tag):
        """dst = src*cos + swap_half(src)*signed_sin, 3 DVE ops."""
        tmp = bwork.tile([128, nheads, HD], BF16_DT, tag=tag)
        tmp4 = tmp[:].rearrange("p h (b d) -> p h b d", b=2)
        nc.vector.tensor_tensor(tmp4, _swap_half(src3), _ss_view(ss_t, nheads),
                                op=mult)
        nc.vector.tensor_tensor(dst3, src3, _bcast(cos_t, nheads), op=mult)
        nc.vector.tensor_add(dst3, dst3, tmp[:])

    def qkv_tile(t):
        b, c0 = t // 4, (t % 4) * 128
        tsl = bass.ts(t, 128)
        psq = psB.tile([128, F_O], FP32, tag="psq", name=f"psq{t}")
        pskv = psB.tile([128, 2 * KVH * HD], FP32, tag="pskv", name=f"pskv{t}")
        for k in range(KT):
            xs = xb[b][:, k, c0:c0 + 128]
            nc.tensor.matmul(psq[:], xs, wk[k][:, 0:F_O],
                             start=(k == 0), stop=(k == KT - 1))
            nc.tensor.matmul(pskv[:], xs, wk[k][:, F_O:F_QKV],
                             start=(k == 0), stop=(k == KT - 1))
        cos_t = cos_sb[:, t, :]
        ss_t = ss_sb[:, t, :]
        # q: 8 heads -> q_nat, then 4 two-head-stacked XBAR transposes
        qn = bwork.tile([128, QH, HD], BF16_DT, tag="qn", name=f"qn{t}")
        rope3(qn[:], psq[:].rearrange("p (h d) -> p h d", d=HD), QH,
              cos_t, ss_t, "qtmp")
        # k: 2 kv heads into half-zeroed staging, then 4 XBAR transposes
        klo, khi = k2lo[t % 2], k2hi[t % 2]
        rope3(klo[:, :, 0:HD],
              pskv[:, 0:KVH * HD].rearrange("p (h d) -> p h d", d=HD), KVH,
              cos_t, ss_t, "ktmp")
        nc.vector.tensor_copy(khi[:, :, HD:128], klo[:, :, 0:HD])
        # v natural + ones column (memset in prologue)
        nc.vector.tensor_copy(
            v_sb[t][:, :, 0:HD],
            pskv[:, KVH * HD:].rearrange("p (h d) -> p h d", d=HD))
        for p in range(4):
            nc.sync.dma_start_transpose(qT[p][:, tsl], qn[:, 2 * p:2 * p + 2, :])
        for j in range(KVH):
            nc.sync.dma_start_transpose(kTlo[:, j, tsl], klo[:, j, :])
            nc.sync.dma_start_transpose(kThi[:, j, tsl], khi[:, j, :])

    # PSUM accumulation-group notes (as in v1): all four O accumulators
    # share one PSUM bank (4x68 fp32). start=True clears the whole bank's
    # has_written state, so ONLY the bank's first matmul (ik=0, j=0)
    # carries start=True; siblings rely on that clear (same-engine order).
    def attn_head(qc, h, fillers):
        tj = [4 * qc + j for j in range(4)]
        n_ik = tj[3] + 1
        jv = h // (QH // KVH)
        kTv = kTlo if h % 2 == 0 else kThi
        qTp = qT[h // 2]
        hsl = bass.ds(h * HD, HD)
        Oall = psO.tile([128, 4, 68], FP32, tag="O", name=f"Op{h}_{qc}")
        Ops = [Oall[:, j, 0:HD + 1] for j in range(4)]
        for g in range(0, n_ik, 2):
            members = [ik for ik in (g, g + 1) if ik < n_ik]
            j0s = [max(0, ik - 4 * qc) for ik in members]
            stp = psS.tile([128, 2, 512], FP32, tag="st")
            p_sb = att.tile([128, 2, 512], BF16_DT, tag="p")
            for m, ik in enumerate(members):
                j0 = j0s[m]
                nc.tensor.matmul(
                    stp[:, m, bass.ds(j0 * 128, 512 - j0 * 128)],
                    kTv[:, jv, bass.ts(ik, 128)],
                    qTp[:, bass.ds(qc * 512 + j0 * 128, 512 - j0 * 128)],
                    start=True, stop=True)
            # one exp over both key tiles, from the leftmost live column on
            jmin = min(j0s)
            ecols = bass.ds(jmin * 128, 512 - jmin * 128)
            nc.scalar.activation(p_sb[:, 0:len(members), ecols],
                                 stp[:, 0:len(members), ecols],
                                 Exp, scale=0.125)
            for m, ik in enumerate(members):
                j0 = j0s[m]
                if ik >= 4 * qc:   # diagonal: mask sub-tile j0
                    nc.vector.tensor_mul(
                        p_sb[:, m, bass.ts(j0, 128)],
                        p_sb[:, m, bass.ts(j0, 128)], mask_sb[:])
                for j in range(j0, 4):
                    nc.tensor.matmul(
                        Ops[j], p_sb[:, m, bass.ts(j, 128)],
                        v_sb[ik][:, jv, :],
                        start=(ik == 0 and j == 0),
                        stop=(ik == tj[j]),
                        skip_group_check=(j > 0))
            if fillers:
                fillers.pop(0)()
        # normalization: one reciprocal + one bcast-multiply for all 4 tiles
        rc4 = fwork.tile([128, 4], FP32, tag="rc")
        nc.vector.reciprocal(rc4[:].unsqueeze(2), Oall[:, :, HD:HD + 1])
        nc.vector.tensor_tensor(o_sb[qc][:, :, hsl], Oall[:, :, 0:HD],
                                _bcast_last(rc4[:], HD), op=mult)

    def o_xbars(qc):
        for j in range(4):
            for f in range(4):
                nc.sync.dma_start_transpose(oT_sb[f][:, bass.ts(j, 128)],
                                            o_sb[qc][:, j, bass.ts(f, 128)])

    def o_mm_group(qc, gi):
        j, nch = divmod(gi, 4)
        t = 4 * qc + j
        po = psF.tile([128, 512], FP32, tag="po")
        for kf in range(4):
            nc.tensor.matmul(po[:], oT_sb[kf][:, bass.ts(j, 128)],
                             wo_sb[:, kf, bass.ts(nch, 512)],
                             start=(kf == 0), stop=(kf == 3))
        ost = fwork.tile([128, 512], FP32, tag="ost")
        nc.vector.tensor_copy(ost[:], po[:])
        nc.gpsimd.dma_start(out=out[bass.ts(t, 128), bass.ts(nch, 512)],
                            in_=ost[:])

    # ---- emission: qkv t-block 0 up front, then attention blocks with
    # qkv(next block) and o_proj(previous block) interleaved as fillers ----
    for t in range(4):
        qkv_tile(t)
    for qc in range(4):
        for h in range(QH):
            fillers = []
            if qc > 0:
                fillers.append(lambda qc=qc, g=2 * h: o_mm_group(qc - 1, g))
                fillers.append(lambda qc=qc, g=2 * h + 1: o_mm_group(qc - 1, g))
            if qc < 3 and h % 2 == 1:
                fillers.append(lambda t=4 * (qc + 1) + h // 2: qkv_tile(t))
            attn_head(qc, h, fillers)
            for f in fillers:
                f()
        o_xbars(qc)
    for gi in range(16):
        o_mm_group(3, gi)
    ctx.close()


_NC_CACHE = None


def _get_nc():
    global _NC_CACHE
    if _NC_CACHE is None:
        _NC_CACHE = _build_nc()
    return _NC_CACHE


def _rope_tables(pos):
    pos = np.asarray(pos, dtype=np.float32)  # [SEQ]
    inv = (1.0 / (np.float32(ROPE_THETA)
                  ** (np.arange(0, HEAD_DIM, 2, dtype=np.float32)
                      / np.float32(HEAD_DIM)))).astype(np.float32)
    fr = pos[:, None] * inv[None, :]                       # [SEQ, 32]
    emb = np.concatenate([fr, fr], axis=-1).astype(np.float32)
    c = np.cos(emb).astype(np.float32)
    s = np.sin(emb).astype(np.float32)
    # signed sin: dst = src*cos + swap_half(src)*ss
    ssg = np.concatenate([-s[:, :32], s[:, 32:]], axis=-1).astype(np.float32)
    return c, ssg


def _make_in_maps(input_ids, Wq, Wk, Wv, Wo, position_ids):
    x = np.asarray(input_ids, dtype=np.float32)
    Wq = np.asarray(Wq, dtype=np.float32)
    Wk = np.asarray(Wk, dtype=np.float32)
    Wv = np.asarray(Wv, dtype=np.float32)
    Wo = np.asarray(Wo, dtype=np.float32)
    pos = np.asarray(position_ids)

    maskt = np.triu(np.ones((128, 128), dtype=np.float32)).astype(BF16)

    in_maps = []
    for c in range(N_CORES):
        b, g = c // TP, c % TP
        xTc = np.ascontiguousarray(x[b].T).astype(BF16)
        wq = Wq[:, g * QH * HEAD_DIM:(g + 1) * QH * HEAD_DIM]
        wk_ = Wk[:, g * KVH * HEAD_DIM:(g + 1) * KVH * HEAD_DIM]
        wv = Wv[:, g * KVH * HEAD_DIM:(g + 1) * KVH * HEAD_DIM]
        wqkv = np.concatenate([wq, wk_, wv], axis=1).astype(BF16)
        wo_s = np.ascontiguousarray(
            Wo[g * F_O:(g + 1) * F_O, :]).astype(BF16)
        cos, ssg = _rope_tables(pos[b])
        in_maps.append({
            "xT": np.ascontiguousarray(xTc),
            "wqkv": np.ascontiguousarray(wqkv),
            "wo": wo_s,
            "cos": cos,
            "ss": ssg,
            "maskt": maskt,
        })
    return in_maps


def _run(in_maps, trace=False):
    nc = _get_nc()
    kwargs = {}
    if trace:
        _install_profile_hook()
        kwargs["trace"] = True
    return run_bass_kernel_spmd(nc, in_maps, core_ids=list(range(N_CORES)),
                                **kwargs)


def _install_profile_hook():
    """This image's antenv lacks axon_hooks; register the NTFF profile hook
    manually so trace=True yields hardware exec times."""
    if "antenv.axon_hooks" in sys.modules:
        return
    import antenv
    mod = types.ModuleType("antenv.axon_hooks")
    state = {"hook": None}
    mod.set_axon_ntff_profile_hook = lambda h: state.__setitem__("hook", h)
    mod.get_axon_ntff_profile_hook = lambda: state["hook"]
    sys.modules["antenv.axon_hooks"] = mod
    antenv.axon_hooks = mod
    try:
        from trn_agent_boot.trn_boot import _ntff_profile_via_ctypes
        mod.set_axon_ntff_profile_hook(
            _ntff_profile_via_ctypes("/opt/axon/libaxon_pjrt.so"))
    except Exception:
        pass


def kernel(input_ids, Wq, Wk, Wv, Wo, position_ids):
    in_maps = _make_in_maps(input_ids, Wq, Wk, Wv, Wo, position_ids)
    res = _run(in_maps, trace=bool(os.environ.get("KERNEL_TRACE")))
    if os.environ.get("KERNEL_TRACE"):
        print(f"HW exec time: {res.exec_time_ns} ns "
              f"(mean {res.mean_exec_time_ns})")
    out = np.zeros((BATCH, SEQ, HIDDEN), dtype=np.float32)
    for c in range(N_CORES):
        out[c // TP] += res.results[c]["out"]
    return out


# revision 4
# speedup vs baseline: 1.4431x; 1.4431x over previous
"""Trainium2 Bass kernel for a dense-transformer attention block.

Module: y = o_proj(causal_sdpa(rope(q_proj(x)), rope(k_proj(x)), v_proj(x)))
Shapes: x [2, 2048, 2048], 32 q heads / 8 kv heads, head_dim 64, fp32 I/O.

Sharding (8 NeuronCores): 2-way data parallel over batch x 4-way tensor
parallel over heads. Core c handles batch c//4 and head group c%4
(8 q heads, 2 kv heads). Each core produces a partial [2048, 2048]
output (its heads' slice of o_proj); the host sums the 4 partials per
batch.

v3 design (evidence-driven; see NTFF analyses of v1/v2):
- q and k are projected DIRECTLY TRANSPOSED: the projection matmuls use
  W slices as the stationary operand and x^T as the moving operand, so
  qT/kT come out [d, seq] with no transpose instructions at all. Head
  pairs (p, p+4) share one stationary (q head p on partitions 0:64,
  p+4 on 64:128) so each q head sits on the same partition half as its
  kv head (matmul requires equal base partitions). The head-dim rows
  are interleaved (d_i, d_{i+32} adjacent) so the RoPE rotate-half
  partner is one partition away, reachable by DVE stream_shuffle
  (which can only permute within 32-partition quadrants). S is
  invariant to this shared row permutation.
- RoPE runs in the transposed layout: stream_shuffle + 3 tensor ops per
  chunk, reading the projection PSUM and writing bf16 SBUF directly.
  k's RoPE writes straight into kTlo ([kv0 | 0]) and kThi ([0 | kv1]);
  the zero halves are memset once. S matmuls then contract K=128 with
  a half-zero stationary — measured: K=64 matmuls run at half clock
  (HAM stays cold), so zero-padding to full K is the fast path.
- v is projected in natural layout (x-chunk stationary) since the O
  matmul needs v [seq, d] as its moving operand.
- Attention per (qc, h): S^T tiles (k stationary, q moving, fp32 PSUM),
  exp on ACT (scale=0.125, no max subtraction: |0.125 S| < ~10), mask
  by upper-triangle multiply on the diagonal tiles, O accumulated with
  P^T stationary / v moving (measured 35ns/matmul in v1), ones-column
  denominator, then ONE reciprocal + ONE broadcast-multiply evict.
- Emission interleaves: attention block qc carries fillers = the qkv
  chunks of block qc+1 and the o_proj groups of block qc-1, so the ACT
  engine's ~190us of exp overlaps PE work across the whole span.
- o transposes (64 total) go through the DMA XBAR on the sync queue
  (only 68 dispatches live there); output DMA via gpsimd SWDGE.
- PSUM: psB (2 bufs x [128,512]f32; q/k/v/o_proj rotation) = 2 banks,
  psS (2 bufs x [128,2,512]f32) = 4, psO (2 bufs x [128,4,68]f32) = 2.
"""

import os
import sys
import types

import numpy as np

sys.path.insert(0, "/opt/trn_rl_repo")

import concourse.bacc as bacc  # noqa: E402
import concourse.bass as bass  # noqa: E402
import concourse.tile as tile  # noqa: E402
from concourse import mybir  # noqa: E402
from concourse.bass_utils import run_bass_kernel_spmd  # noqa: E402

try:
    import ml_dtypes
    BF16 = ml_dtypes.bfloat16
except ImportError:  # pragma: no cover
    BF16 = np.dtype("bfloat16")

HIDDEN = 2048
SEQ = 2048
BATCH = 2
N_HEADS = 32
N_KV_HEADS = 8
HEAD_DIM = 64
ROPE_THETA = 10000.0

N_CORES = 8
TP = 4                      # head-parallel ways
QH = N_HEADS // TP          # 8 q heads per core
KVH = N_KV_HEADS // TP      # 2 kv heads per core
KT = HIDDEN // 128          # 16 contraction tiles
TT = SEQ // 128             # 16 seq tiles
HD = HEAD_DIM
F_QT = 512                  # 4 pair-stacked qT stationary column blocks
F_KT = 128                  # 1 stacked kT stationary column block
F_V = 128                   # v natural columns (2 kv heads x 64)
F_W = F_QT + F_KT + F_V     # 768
F_O = QH * HEAD_DIM         # 512

FP32 = mybir.dt.float32
BF16_DT = mybir.dt.bfloat16

SHUF_MASK = [i ^ 1 for i in range(32)]


def _build_nc():
    nc = bacc.Bacc("TRN2", target_bir_lowering=False, debug=False)

    xT = nc.dram_tensor("xT", [HIDDEN, SEQ], BF16_DT, kind="ExternalInput")
    wall = nc.dram_tensor("wall", [HIDDEN, F_W], BF16_DT, kind="ExternalInput")
    wo = nc.dram_tensor("wo", [F_O, HIDDEN], BF16_DT, kind="ExternalInput")
    cosT = nc.dram_tensor("cosT", [128, SEQ], FP32, kind="ExternalInput")
    ssT = nc.dram_tensor("ssT", [128, SEQ], FP32, kind="ExternalInput")
    maskt = nc.dram_tensor("maskt", [128, 128], BF16_DT, kind="ExternalInput")
    out = nc.dram_tensor("out", [SEQ, HIDDEN], FP32, kind="ExternalOutput")

    with tile.TileContext(nc) as tc:
        _emit(nc, tc, xT, wall, wo, cosT, ssT, maskt, out)
    nc.compile()
    return nc


def _bcast_last(ap, n):
    """Append a step-0 (broadcast) dim of size n to an AP."""
    return bass.AP(tensor=ap.tensor, offset=ap.offset, ap=list(ap.ap) + [[0, n]])


def _emit(nc, tc, xT, wall, wo, cosT, ssT, maskt, out):
    from contextlib import ExitStack
    ctx = ExitStack()
    Exp = mybir.ActivationFunctionType.Exp
    mult = mybir.AluOpType.mult

    const = ctx.enter_context(tc.tile_pool(name="const", bufs=1))
    persist = ctx.enter_context(tc.tile_pool(name="persist", bufs=1))

    # ---- weights/tables on the scalar HWDGE queue ----
    wk = [const.tile([128, F_W], BF16_DT, name=f"wk{k}") for k in range(KT)]
    w_r = wall[:].rearrange("(k p) f -> p k f", p=128)
    for k in range(KT):
        nc.scalar.dma_start(out=wk[k][:], in_=w_r[:, k, :])
    cosT2 = const.tile([128, SEQ], FP32)
    ssT2 = const.tile([128, SEQ], FP32)
    nc.scalar.dma_start(out=cosT2[:], in_=cosT[:])
    nc.scalar.dma_start(out=ssT2[:], in_=ssT[:])
    mask_sb = const.tile([128, 128], BF16_DT)
    nc.scalar.dma_start(out=mask_sb[:], in_=maskt[:])
    wo_sb = const.tile([128, F_O // 128, HIDDEN], BF16_DT)
    nc.scalar.dma_start(out=wo_sb[:], in_=wo[:].rearrange("(k p) d -> p k d", p=128))

    # ---- x as 4 per-t-block tensors on the sync HWDGE queue ----
    xb = [const.tile([128, KT, 512], BF16_DT, name=f"xb{b}") for b in range(4)]
    xT_r = xT[:].rearrange("(k p) t -> p k t", p=128)
    for b in range(4):
        nc.sync.dma_start(out=xb[b][:], in_=xT_r[:, :, bass.ts(b, 512)])

    # ---- persistent SBUF tensors ----
    qT = [persist.tile([128, SEQ], BF16_DT, name=f"qT{p}") for p in range(4)]
    kTlo = persist.tile([128, SEQ], BF16_DT, name="kTlo")   # [kv0 | zeros]
    kThi = persist.tile([128, SEQ], BF16_DT, name="kThi")   # [zeros | kv1]
    v_sb = [persist.tile([128, 4, KVH, HD + 1], BF16_DT, name=f"v{b}")
            for b in range(4)]
    o_sb = [persist.tile([128, 4, F_O], BF16_DT, name=f"ob{qc}") for qc in range(4)]
    oT_sb = [persist.tile([128, 512], BF16_DT, name=f"oT{f}") for f in range(4)]
    nc.gpsimd.memset(kTlo[64:128, :], 0.0)
    nc.gpsimd.memset(kThi[0:64, :], 0.0)
    for b in range(4):
        nc.gpsimd.memset(v_sb[b][:, :, :, HD:HD + 1], 1.0)

    # ---- pools ----
    psB = ctx.enter_context(tc.tile_pool(name="psB", bufs=2, space="PSUM"))
    psS = ctx.enter_context(tc.tile_pool(name="psS", bufs=2, space="PSUM"))
    psO = ctx.enter_context(tc.tile_pool(name="psO", bufs=2, space="PSUM"))
    bwork = ctx.enter_context(tc.tile_pool(name="bwork", bufs=2))
    att = ctx.enter_context(tc.tile_pool(name="att", bufs=6))
    fwork = ctx.enter_context(tc.tile_pool(name="fwork", bufs=3))

    def q_chunk(b, p):
        """Pair-stacked transposed q projection: heads (p, p+4), 512 s-cols."""
        bcols = bass.ds(b * 512, 512)
        ps = psB.tile([128, 512], FP32, tag="pb", name=f"q{b}_{p}")
        for k in range(KT):
            nc.tensor.matmul(ps[:], wk[k][:, bass.ts(p, 128)], xb[b][:, k, :],
                             start=(k == 0), stop=(k == KT - 1))
        shf = bwork.tile([128, 512], FP32, tag="shf")
        tm = bwork.tile([128, 512], BF16_DT, tag="tm")
        nc.vector.stream_shuffle(shf[:], ps[:], mask=SHUF_MASK)
        nc.vector.tensor_tensor(tm[:], shf[:], ssT2[:, bcols], op=mult)
        dst = qT[p][:, bcols]
        nc.vector.tensor_tensor(dst, ps[:], cosT2[:, bcols], op=mult)
        nc.vector.tensor_add(dst, dst, tm[:])

    def k_chunk(b):
        """Stacked transposed k projection -> kTlo[0:64], kThi[64:128]."""
        bcols = bass.ds(b * 512, 512)
        ps = psB.tile([128, 512], FP32, tag="pb", name=f"k{b}")
        for k in range(KT):
            nc.tensor.matmul(ps[:], wk[k][:, F_QT:F_QT + 128], xb[b][:, k, :],
                             start=(k == 0), stop=(k == KT - 1))
        shf = bwork.tile([128, 512], FP32, tag="shf")
        tm = bwork.tile([128, 512], BF16_DT, tag="tm")
        nc.vector.stream_shuffle(shf[:], ps[:], mask=SHUF_MASK)
        nc.vector.tensor_tensor(tm[:], shf[:], ssT2[:, bcols], op=mult)
        for lo, hi, kt in ((0, 64, kTlo), (64, 128, kThi)):
            dst = kt[lo:hi, bcols]
            nc.vector.tensor_tensor(dst, ps[lo:hi, :], cosT2[lo:hi, bcols],
                                    op=mult)
            nc.vector.tensor_add(dst, dst, tm[lo:hi, :])

    def v_block(b):
        """Natural-layout v projection for the 4 seq tiles of block b."""
        ps = psB.tile([128, 512], FP32, tag="pb", name=f"v{b}")
        pv = ps[:].rearrange("p (j f) -> p j f", f=128)
        for j in range(4):
            for k in range(KT):
                nc.tensor.matmul(pv[:, j, :],
                                 xb[b][:, k, bass.ts(j, 128)],
                                 wk[k][:, F_QT + F_KT:F_W],
                                 start=(j == 0 and k == 0), stop=(k == KT - 1),
                                 skip_group_check=(j > 0))
        nc.vector.tensor_copy(
            v_sb[b][:, :, :, 0:HD],
            pv.rearrange("p j (h d) -> p j h d", d=HD))

    # PSUM accumulation-group notes: all four O accumulators share one
    # bank (4x68 fp32). start=True clears the whole bank's has_written
    # state, so ONLY the bank's first matmul (ik=0, j=0) carries
    # start=True; siblings rely on that clear (same-engine order).
    def attn_head(qc, h, fillers):
        tj = [4 * qc + j for j in range(4)]
        n_ik = tj[3] + 1
        jv = h // 4
        kTv = kTlo if h < 4 else kThi
        qTp = qT[h % 4]
        hsl = bass.ds(h * HD, HD)
        Oall = psO.tile([128, 4, 68], FP32, tag="O", name=f"Op{h}_{qc}")
        Ops = [Oall[:, j, 0:HD + 1] for j in range(4)]
        for g in range(0, n_ik, 2):
            members = [ik for ik in (g, g + 1) if ik < n_ik]
            j0s = [max(0, ik - 4 * qc) for ik in members]
            stp = psS.tile([128, 2, 512], FP32, tag="st")
            p_sb = att.tile([128, 2, 512], BF16_DT, tag="p")
            for m, ik in enumerate(members):
                j0 = j0s[m]
                nc.tensor.matmul(
                    stp[:, m, bass.ds(j0 * 128, 512 - j0 * 128)],
                    kTv[:, bass.ts(ik, 128)],
                    qTp[:, bass.ds(qc * 512 + j0 * 128, 512 - j0 * 128)],
                    start=True, stop=True)
            # one exp over both key tiles, from the leftmost live column on
            jmin = min(j0s)
            ecols = bass.ds(jmin * 128, 512 - jmin * 128)
            nc.scalar.activation(p_sb[:, 0:len(members), ecols],
                                 stp[:, 0:len(members), ecols],
                                 Exp, scale=0.125)
            for m, ik in enumerate(members):
                j0 = j0s[m]
                if ik >= 4 * qc:   # diagonal: mask sub-tile j0
                    nc.vector.tensor_mul(
                        p_sb[:, m, bass.ts(j0, 128)],
                        p_sb[:, m, bass.ts(j0, 128)], mask_sb[:])
                for j in range(j0, 4):
                    nc.tensor.matmul(
                        Ops[j], p_sb[:, m, bass.ts(j, 128)],
                        v_sb[ik // 4][:, ik % 4, jv, :],
                        start=(ik == 0 and j == 0),
                        stop=(ik == tj[j]),
                        skip_group_check=(j > 0))
            if fillers:
                fillers.pop(0)()
        # normalization: one reciprocal + one bcast-multiply for all 4 tiles
        rc4 = fwork.tile([128, 4], FP32, tag="rc")
        nc.vector.reciprocal(rc4[:].unsqueeze(2), Oall[:, :, HD:HD + 1])
        nc.vector.tensor_tensor(o_sb[qc][:, :, hsl], Oall[:, :, 0:HD],
                                _bcast_last(rc4[:], HD), op=mult)
        for f in fillers:
            f()

    def o_xbars(qc):
        for j in range(4):
            for f in range(4):
                nc.sync.dma_start_transpose(oT_sb[f][:, bass.ts(j, 128)],
                                            o_sb[qc][:, j, bass.ts(f, 128)])

    def o_mm_group(qc, gi):
        j, nch = divmod(gi, 4)
        t = 4 * qc + j
        po = psB.tile([128, 512], FP32, tag="pb", name=f"po{qc}_{gi}")
        for kf in range(4):
            nc.tensor.matmul(po[:], oT_sb[kf][:, bass.ts(j, 128)],
                             wo_sb[:, kf, bass.ts(nch, 512)],
                             start=(kf == 0), stop=(kf == 3))
        ost = fwork.tile([128, 512], FP32, tag="ost")
        nc.vector.tensor_copy(ost[:], po[:])
        nc.gpsimd.dma_start(out=out[bass.ts(t, 128), bass.ts(nch, 512)],
                            in_=ost[:])

    def qkv_fns(b):
        fns = [lambda p=p: q_chunk(b, p) for p in range(4)]
        fns.append(lambda: k_chunk(b))
        fns.append(lambda: v_block(b))
        return fns

    # ---- emission ----
    for f in qkv_fns(0):
        f()
    for qc in range(4):
        # fillers: qkv of next block first (next attention block needs it),
        # then o_proj groups of the previous block, j-major so each group
        # follows its XBARs with slack.
        fill = []
        if qc < 3:
            fill += qkv_fns(qc + 1)
        if qc > 0:
            fill += [lambda qc=qc, g=g: o_mm_group(qc - 1, g) for g in range(16)]
        # distribute sequentially (qkv first, then o_proj j-major) so each
        # o_proj group trails its XBAR dispatches; for the o_proj-only last
        # block start at head 1 so head 0 never waits on fresh XBARs.
        heads = list(range(QH)) if qc < 3 else list(range(1, QH))
        per_head = {h: [] for h in range(QH)}
        if fill:
            q, r = divmod(len(fill), len(heads))
            i = 0
            for n, h in enumerate(heads):
                take = q + (1 if n < r else 0)
                per_head[h] = fill[i:i + take]
                i += take
        for h in range(QH):
            attn_head(qc, h, per_head[h])
        o_xbars(qc)
    for g in range(16):
        o_mm_group(3, g)
    ctx.close()


_NC_CACHE = None


def _get_nc():
    global _NC_CACHE
    if _NC_CACHE is None:
        _NC_CACHE = _build_nc()
    return _NC_CACHE


# interleaved head-dim order: row 2i = d_i, row 2i+1 = d_{i+32}
_PHI = np.empty(64, dtype=np.int64)
_PHI[0::2] = np.arange(32)
_PHI[1::2] = np.arange(32) + 32


def _rope_tables_T(pos):
    """Transposed rope tables in the interleaved row order, [128, SEQ]."""
    pos = np.asarray(pos, dtype=np.float32)
    inv = (1.0 / (np.float32(ROPE_THETA)
                  ** (np.arange(0, HEAD_DIM, 2, dtype=np.float32)
                      / np.float32(HEAD_DIM)))).astype(np.float32)  # [32]
    # row r (within 64): dim pair index i = r//2; angle = pos * inv[i]
    ang = inv[(np.arange(64) // 2)][:, None] * pos[None, :]   # [64, SEQ]
    c = np.cos(ang)
    s = np.sin(ang)
    sign = np.where(np.arange(64) % 2 == 0, -1.0, 1.0).astype(np.float32)
    ss = s * sign[:, None]
    cosT = np.concatenate([c, c], axis=0).astype(np.float32)   # [128, SEQ]
    ssT = np.concatenate([ss, ss], axis=0).astype(np.float32)
    return cosT, ssT


def _make_in_maps(input_ids, Wq, Wk, Wv, Wo, position_ids):
    x = np.asarray(input_ids, dtype=np.float32)
    Wq = np.asarray(Wq, dtype=np.float32)
    Wk = np.asarray(Wk, dtype=np.float32)
    Wv = np.asarray(Wv, dtype=np.float32)
    Wo = np.asarray(Wo, dtype=np.float32)
    pos = np.asarray(position_ids)

    maskt = np.triu(np.ones((128, 128), dtype=np.float32)).astype(BF16)

    in_maps = []
    for c in range(N_CORES):
        b, g = c // TP, c % TP
        xTc = np.ascontiguousarray(x[b].T).astype(BF16)
        # q pair-stacked stationaries: pair p = local heads (p, p+4),
        # columns phi-permuted within each head
        qcols = []
        for p in range(4):
            for hh in (p, p + 4):
                base = (g * QH + hh) * HEAD_DIM
                qcols.extend((base + _PHI).tolist())
        wq_t = Wq[:, qcols]                                    # [H, 512]
        # k stacked stationary: kv0 then kv1, phi-permuted
        kcols = []
        for j in range(KVH):
            base = (g * KVH + j) * HEAD_DIM
            kcols.extend((base + _PHI).tolist())
        wk_t = Wk[:, kcols]                                    # [H, 128]
        # v natural
        wv_n = Wv[:, g * KVH * HEAD_DIM:(g + 1) * KVH * HEAD_DIM]
        wall = np.concatenate([wq_t, wk_t, wv_n], axis=1).astype(BF16)
        wo_s = np.ascontiguousarray(
            Wo[g * F_O:(g + 1) * F_O, :]).astype(BF16)
        cosT, ssT = _rope_tables_T(pos[b])
        in_maps.append({
            "xT": np.ascontiguousarray(xTc),
            "wall": np.ascontiguousarray(wall),
            "wo": wo_s,
            "cosT": cosT,
            "ssT": ssT,
            "maskt": maskt,
        })
    return in_maps


def _run(in_maps, trace=False):
    nc = _get_nc()
    kwargs = {}
    if trace:
        _install_profile_hook()
        kwargs["trace"] = True
    return run_bass_kernel_spmd(nc, in_maps, core_ids=list(range(N_CORES)),
                                **kwargs)


def _install_profile_hook():
    """This image's antenv lacks axon_hooks; register the NTFF profile hook
    manually so trace=True yields hardware exec times."""
    if "antenv.axon_hooks" in sys.modules:
        return
    import antenv
    mod = types.ModuleType("antenv.axon_hooks")
    state = {"hook": None}
    mod.set_axon_ntff_profile_hook = lambda h: state.__setitem__("hook", h)
    mod.get_axon_ntff_profile_hook = lambda: state["hook"]
    sys.modules["antenv.axon_hooks"] = mod
    antenv.axon_hooks = mod
    try:
        from trn_agent_boot.trn_boot import _ntff_profile_via_ctypes
        mod.set_axon_ntff_profile_hook(
            _ntff_profile_via_ctypes("/opt/axon/libaxon_pjrt.so"))
    except Exception:
        pass


def kernel(input_ids, Wq, Wk, Wv, Wo, position_ids):
    in_maps = _make_in_maps(input_ids, Wq, Wk, Wv, Wo, position_ids)
    res = _run(in_maps, trace=bool(os.environ.get("KERNEL_TRACE")))
    if os.environ.get("KERNEL_TRACE"):
        print(f"HW exec time: {res.exec_time_ns} ns "
              f"(mean {res.mean_exec_time_ns})")
    out = np.zeros((BATCH, SEQ, HIDDEN), dtype=np.float32)
    for c in range(N_CORES):
        out[c // TP] += res.results[c]["out"]
    return out


# revision 11
# speedup vs baseline: 1.4474x; 1.0030x over previous
"""Trainium2 Bass kernel for a dense-transformer attention block.

Module: y = o_proj(causal_sdpa(rope(q_proj(x)), rope(k_proj(x)), v_proj(x)))
Shapes: x [2, 2048, 2048], 32 q heads / 8 kv heads, head_dim 64, fp32 I/O.

Sharding (8 NeuronCores): 2-way data parallel over batch x 4-way tensor
parallel over heads. Core c handles batch c//4 and head group c%4
(8 q heads, 2 kv heads). Each core produces a partial [2048, 2048]
output (its heads' slice of o_proj); the host sums the 4 partials per
batch.

v3 design (evidence-driven; see NTFF analyses of v1/v2):
- q and k are projected DIRECTLY TRANSPOSED: the projection matmuls use
  W slices as the stationary operand and x^T as the moving operand, so
  qT/kT come out [d, seq] with no transpose instructions at all. Head
  pairs (p, p+4) share one stationary (q head p on partitions 0:64,
  p+4 on 64:128) so each q head sits on the same partition half as its
  kv head (matmul requires equal base partitions). The head-dim rows
  are interleaved (d_i, d_{i+32} adjacent) so the RoPE rotate-half
  partner is one partition away, reachable by DVE stream_shuffle
  (which can only permute within 32-partition quadrants). S is
  invariant to this shared row permutation.
- RoPE runs in the transposed layout: stream_shuffle + 3 tensor ops per
  chunk, reading the projection PSUM and writing bf16 SBUF directly.
  k's RoPE writes straight into kTlo ([kv0 | 0]) and kThi ([0 | kv1]);
  the zero halves are memset once. S matmuls then contract K=128 with
  a half-zero stationary — measured: K=64 matmuls run at half clock
  (HAM stays cold), so zero-padding to full K is the fast path.
- v is projected in natural layout (x-chunk stationary) since the O
  matmul needs v [seq, d] as its moving operand.
- Attention per (qc, h): S^T tiles (k stationary, q moving, fp32 PSUM),
  exp on ACT (scale=0.125, no max subtraction: |0.125 S| < ~10), mask
  by upper-triangle multiply on the diagonal tiles, O accumulated with
  P^T stationary / v moving (measured 35ns/matmul in v1), ones-column
  denominator, then ONE reciprocal + ONE broadcast-multiply evict.
- Emission interleaves: attention block qc carries fillers = the qkv
  chunks of block qc+1 and the o_proj groups of block qc-1, so the ACT
  engine's ~190us of exp overlaps PE work across the whole span.
- o transposes (64 total) go through the DMA XBAR on the sync queue
  (only 68 dispatches live there); output DMA via gpsimd SWDGE.
- PSUM: psB (2 bufs x [128,512]f32; q/k/v/o_proj rotation) = 2 banks,
  psS (2 bufs x [128,2,512]f32) = 4, psO (2 bufs x [128,4,68]f32) = 2.
"""

import os
import sys
import types

import numpy as np

sys.path.insert(0, "/opt/trn_rl_repo")

import concourse.bacc as bacc  # noqa: E402
import concourse.bass as bass  # noqa: E402
import concourse.tile as tile  # noqa: E402
from concourse import mybir  # noqa: E402
from concourse.bass_utils import run_bass_kernel_spmd  # noqa: E402

try:
    import ml_dtypes
    BF16 = ml_dtypes.bfloat16
except ImportError:  # pragma: no cover
    BF16 = np.dtype("bfloat16")

HIDDEN = 2048
SEQ = 2048
BATCH = 2
N_HEADS = 32
N_KV_HEADS = 8
HEAD_DIM = 64
ROPE_THETA = 10000.0

N_CORES = 8
TP = 4                      # head-parallel ways
QH = N_HEADS // TP          # 8 q heads per core
KVH = N_KV_HEADS // TP      # 2 kv heads per core
KT = HIDDEN // 128          # 16 contraction tiles
TT = SEQ // 128             # 16 seq tiles
HD = HEAD_DIM
F_QT = 512                  # 4 pair-stacked qT stationary column blocks
F_KT = 128                  # 1 stacked kT stationary column block
F_V = 128                   # v natural columns (2 kv heads x 64)
F_W = F_QT + F_KT + F_V     # 768
F_O = QH * HEAD_DIM         # 512

FP32 = mybir.dt.float32
BF16_DT = mybir.dt.bfloat16

SHUF_MASK = [i ^ 1 for i in range(32)]


def _build_nc():
    nc = bacc.Bacc("TRN2", target_bir_lowering=False, debug=False)

    xT = nc.dram_tensor("xT", [HIDDEN, SEQ], BF16_DT, kind="ExternalInput")
    wall = nc.dram_tensor("wall", [HIDDEN, F_W], BF16_DT, kind="ExternalInput")
    wo = nc.dram_tensor("wo", [F_O, HIDDEN], BF16_DT, kind="ExternalInput")
    cosT = nc.dram_tensor("cosT", [128, SEQ], FP32, kind="ExternalInput")
    ssT = nc.dram_tensor("ssT", [128, SEQ], FP32, kind="ExternalInput")
    maskt = nc.dram_tensor("maskt", [128, 128], BF16_DT, kind="ExternalInput")
    out = nc.dram_tensor("out", [SEQ, HIDDEN], BF16_DT, kind="ExternalOutput")

    with tile.TileContext(nc) as tc:
        _emit(nc, tc, xT, wall, wo, cosT, ssT, maskt, out)
    nc.compile()
    return nc


def _bcast_last(ap, n):
    """Append a step-0 (broadcast) dim of size n to an AP."""
    return bass.AP(tensor=ap.tensor, offset=ap.offset, ap=list(ap.ap) + [[0, n]])


def _emit(nc, tc, xT, wall, wo, cosT, ssT, maskt, out):
    from contextlib import ExitStack
    ctx = ExitStack()
    Exp = mybir.ActivationFunctionType.Exp
    mult = mybir.AluOpType.mult

    const = ctx.enter_context(tc.tile_pool(name="const", bufs=1))
    persist = ctx.enter_context(tc.tile_pool(name="persist", bufs=1))

    # ---- weights/tables on the scalar HWDGE queue ----
    wk = [const.tile([128, F_W], BF16_DT, name=f"wk{k}") for k in range(KT)]
    w_r = wall[:].rearrange("(k p) f -> p k f", p=128)
    for k in range(KT):
        nc.scalar.dma_start(out=wk[k][:], in_=w_r[:, k, :])
    cosT2 = const.tile([128, SEQ], FP32)
    ssT2 = const.tile([128, SEQ], FP32)
    nc.scalar.dma_start(out=cosT2[:], in_=cosT[:])
    nc.scalar.dma_start(out=ssT2[:], in_=ssT[:])
    mask_sb = const.tile([128, 128], BF16_DT)
    nc.scalar.dma_start(out=mask_sb[:], in_=maskt[:])
    wo_sb = const.tile([128, F_O // 128, HIDDEN], BF16_DT)
    nc.scalar.dma_start(out=wo_sb[:], in_=wo[:].rearrange("(k p) d -> p k d", p=128))

    # ---- x as 4 per-t-block tensors on the sync HWDGE queue ----
    xb = [const.tile([128, KT, 512], BF16_DT, name=f"xb{b}") for b in range(4)]
    xT_r = xT[:].rearrange("(k p) t -> p k t", p=128)
    for b in range(4):
        nc.sync.dma_start(out=xb[b][:], in_=xT_r[:, :, bass.ts(b, 512)])

    # ---- persistent SBUF tensors ----
    qT = [persist.tile([128, SEQ], BF16_DT, name=f"qT{p}") for p in range(4)]
    kTlo = persist.tile([128, SEQ], BF16_DT, name="kTlo")   # [kv0 | zeros]
    kThi = persist.tile([128, SEQ], BF16_DT, name="kThi")   # [zeros | kv1]
    v_sb = [persist.tile([128, 4, KVH, HD + 1], BF16_DT, name=f"v{b}")
            for b in range(4)]
    o_sb = [persist.tile([128, 4, F_O], BF16_DT, name=f"ob{qc}") for qc in range(4)]
    # 3-parity oT buffers: o_mm(qc) reads oT[qc % 3] while incremental
    # XBARs for later blocks write the other parities.
    oT_sb = [[persist.tile([128, 512], BF16_DT, name=f"oT{par}_{f}")
              for f in range(4)] for par in range(3)]
    nc.gpsimd.memset(kTlo[64:128, :], 0.0)
    nc.gpsimd.memset(kThi[0:64, :], 0.0)
    for b in range(4):
        nc.gpsimd.memset(v_sb[b][:, :, :, HD:HD + 1], 1.0)

    # ---- pools ----
    psB = ctx.enter_context(tc.tile_pool(name="psB", bufs=2, space="PSUM"))
    psS = ctx.enter_context(tc.tile_pool(name="psS", bufs=2, space="PSUM"))
    psO = ctx.enter_context(tc.tile_pool(name="psO", bufs=2, space="PSUM"))
    bwork = ctx.enter_context(tc.tile_pool(name="bwork", bufs=2))
    att = ctx.enter_context(tc.tile_pool(name="att", bufs=6))
    fwork = ctx.enter_context(tc.tile_pool(name="fwork", bufs=3))

    def q_chunk(b, p):
        """Pair-stacked transposed q projection: heads (p, p+4), 512 s-cols."""
        bcols = bass.ds(b * 512, 512)
        ps = psB.tile([128, 512], FP32, tag="pb", name=f"q{b}_{p}")
        for k in range(KT):
            nc.tensor.matmul(ps[:], wk[k][:, bass.ts(p, 128)], xb[b][:, k, :],
                             start=(k == 0), stop=(k == KT - 1))
        shf = bwork.tile([128, 512], FP32, tag="shf")
        tm = bwork.tile([128, 512], BF16_DT, tag="tm")
        dst = qT[p][:, bcols]
        # ps-reading ops first so the PSUM slot frees as early as possible
        nc.vector.stream_shuffle(shf[:], ps[:], mask=SHUF_MASK)
        nc.vector.tensor_tensor(dst, ps[:], cosT2[:, bcols], op=mult)
        nc.vector.tensor_tensor(tm[:], shf[:], ssT2[:, bcols], op=mult)
        nc.vector.tensor_add(dst, dst, tm[:])

    def k_chunk(b):
        """Stacked transposed k projection -> kTlo[0:64], kThi[64:128]."""
        bcols = bass.ds(b * 512, 512)
        ps = psB.tile([128, 512], FP32, tag="pb", name=f"k{b}")
        for k in range(KT):
            nc.tensor.matmul(ps[:], wk[k][:, F_QT:F_QT + 128], xb[b][:, k, :],
                             start=(k == 0), stop=(k == KT - 1))
        shf = bwork.tile([128, 512], FP32, tag="shf")
        tm = bwork.tile([128, 512], BF16_DT, tag="tm")
        nc.vector.stream_shuffle(shf[:], ps[:], mask=SHUF_MASK)
        for lo, hi, kt in ((0, 64, kTlo), (64, 128, kThi)):
            nc.vector.tensor_tensor(kt[lo:hi, bcols], ps[lo:hi, :],
                                    cosT2[lo:hi, bcols], op=mult)
        nc.vector.tensor_tensor(tm[:], shf[:], ssT2[:, bcols], op=mult)
        for lo, hi, kt in ((0, 64, kTlo), (64, 128, kThi)):
            dst = kt[lo:hi, bcols]
            nc.vector.tensor_add(dst, dst, tm[lo:hi, :])

    def v_block(b):
        """Natural-layout v projection for the 4 seq tiles of block b."""
        ps = psB.tile([128, 512], FP32, tag="pb", name=f"v{b}")
        pv = ps[:].rearrange("p (j f) -> p j f", f=128)
        for j in range(4):
            for k in range(KT):
                nc.tensor.matmul(pv[:, j, :],
                                 xb[b][:, k, bass.ts(j, 128)],
                                 wk[k][:, F_QT + F_KT:F_W],
                                 start=(j == 0 and k == 0), stop=(k == KT - 1),
                                 skip_group_check=(j > 0))
        nc.vector.tensor_copy(
            v_sb[b][:, :, :, 0:HD],
            pv.rearrange("p j (h d) -> p j h d", d=HD))

    # PSUM accumulation-group notes: all four O accumulators share one
    # bank (4x68 fp32). start=True clears the whole bank's has_written
    # state, so ONLY the bank's first matmul (ik=0, j=0) carries
    # start=True; siblings rely on that clear (same-engine order).
    def attn_head(qc, h, fillers):
        tj = [4 * qc + j for j in range(4)]
        n_ik = tj[3] + 1
        jv = h // 4
        kTv = kTlo if h < 4 else kThi
        qTp = qT[h % 4]
        hsl = bass.ds(h * HD, HD)
        Oall = psO.tile([128, 4, 68], FP32, tag="O", name=f"Op{h}_{qc}")
        Ops = [Oall[:, j, 0:HD + 1] for j in range(4)]
        for g in range(0, n_ik, 2):
            members = [ik for ik in (g, g + 1) if ik < n_ik]
            j0s = [max(0, ik - 4 * qc) for ik in members]
            stp = psS.tile([128, 2, 512], FP32, tag="st")
            p_sb = att.tile([128, 2, 512], BF16_DT, tag="p")
            for m, ik in enumerate(members):
                j0 = j0s[m]
                nc.tensor.matmul(
                    stp[:, m, bass.ds(j0 * 128, 512 - j0 * 128)],
                    kTv[:, bass.ts(ik, 128)],
                    qTp[:, bass.ds(qc * 512 + j0 * 128, 512 - j0 * 128)],
                    start=True, stop=True)
            # one exp over both key tiles, from the leftmost live column on
            jmin = min(j0s)
            ecols = bass.ds(jmin * 128, 512 - jmin * 128)
            nc.scalar.activation(p_sb[:, 0:len(members), ecols],
                                 stp[:, 0:len(members), ecols],
                                 Exp, scale=0.125)
            for m, ik in enumerate(members):
                j0 = j0s[m]
                if ik >= 4 * qc:   # diagonal: mask sub-tile j0 (on gpsimd to
                    # keep the DVE FIFO short — PE stalls trace to DVE waits)
                    nc.gpsimd.tensor_mul(
                        p_sb[:, m, bass.ts(j0, 128)],
                        p_sb[:, m, bass.ts(j0, 128)], mask_sb[:])
                for j in range(j0, 4):
                    nc.tensor.matmul(
                        Ops[j], p_sb[:, m, bass.ts(j, 128)],
                        v_sb[ik // 4][:, ik % 4, jv, :],
                        start=(ik == 0 and j == 0),
                        stop=(ik == tj[j]),
                        skip_group_check=(j > 0))
            if fillers:
                fillers.pop(0)()
        # normalization: one reciprocal + one bcast-multiply for all 4 tiles
        rc4 = fwork.tile([128, 4], FP32, tag="rc")
        nc.vector.reciprocal(rc4[:].unsqueeze(2), Oall[:, :, HD:HD + 1])
        nc.vector.tensor_tensor(o_sb[qc][:, :, hsl], Oall[:, :, 0:HD],
                                _bcast_last(rc4[:], HD), op=mult)
        for f in fillers:
            f()

    def o_xbars_f(qc, f):
        """XBAR-transpose feature columns f of block qc (needs heads 2f,2f+1)."""
        for j in range(4):
            nc.sync.dma_start_transpose(oT_sb[qc % 3][f][:, bass.ts(j, 128)],
                                        o_sb[qc][:, j, bass.ts(f, 128)])

    def o_mm_group(qc, gi):
        j, nch = divmod(gi, 4)
        t = 4 * qc + j
        po = psB.tile([128, 512], FP32, tag="pb", name=f"po{qc}_{gi}")
        for kf in range(4):
            nc.tensor.matmul(po[:], oT_sb[qc % 3][kf][:, bass.ts(j, 128)],
                             wo_sb[:, kf, bass.ts(nch, 512)],
                             start=(kf == 0), stop=(kf == 3))
        ost = fwork.tile([128, 512], BF16_DT, tag="ost")
        nc.vector.tensor_copy(ost[:], po[:])
        nc.gpsimd.dma_start(out=out[bass.ts(t, 128), bass.ts(nch, 512)],
                            in_=ost[:])

    # ---- emission ----
    # Attention item order: blocks 0, 1 whole; then qc2 h0-3; then qc3 heads
    # zipped with qc2 h4-7 (balances the exp-heavy late blocks against PE
    # work); then qc3 h4-7. Fillers (qkv chunks of the next block, o_proj
    # groups of finished blocks) are attached per item, qkv earliest.
    def om(qc, g):
        return lambda: o_mm_group(qc, g)

    qk = {b: ([lambda b=b, p=p: q_chunk(b, p) for p in range(4)]
              + [lambda b=b: k_chunk(b), lambda b=b: v_block(b)])
          for b in range(1, 4)}

    items = []   # (qc, h, fillers)
    items += [(0, 0, qk[1][0:2]), (0, 1, qk[1][2:4]), (0, 2, [qk[1][4]]),
              (0, 3, [qk[1][5]]), (0, 4, []), (0, 5, []), (0, 6, []),
              (0, 7, [])]
    items += [(1, 0, qk[2][0:2]), (1, 1, qk[2][2:4]), (1, 2, qk[2][4:6]),
              (1, 3, [om(0, g) for g in range(0, 3)]),
              (1, 4, [om(0, g) for g in range(3, 6)]),
              (1, 5, [om(0, g) for g in range(6, 9)]),
              (1, 6, [om(0, g) for g in range(9, 13)]),
              (1, 7, [om(0, g) for g in range(13, 16)])]
    items += [(2, 0, qk[3][4:6]), (2, 1, qk[3][0:2]), (2, 2, qk[3][2:4]),
              (2, 3, [om(1, g) for g in range(0, 2)])]
    items += [(3, 0, []), (2, 4, [om(1, g) for g in range(2, 6)]),
              (3, 1, []), (2, 5, [om(1, g) for g in range(6, 10)]),
              (3, 2, []), (2, 6, [om(1, g) for g in range(10, 13)]),
              (3, 3, []), (2, 7, [om(1, g) for g in range(13, 16)])]
    items += [(3, 4, [om(2, g) for g in range(0, 4)]),
              (3, 5, [om(2, g) for g in range(4, 8)]),
              (3, 6, [om(2, g) for g in range(8, 12)]),
              (3, 7, [om(2, g) for g in range(12, 16)])]

    q_chunk(0, 0)
    q_chunk(0, 1)
    k_chunk(0)
    q_chunk(0, 2)
    q_chunk(0, 3)
    v_block(0)
    for qc, h, fillers in items:
        attn_head(qc, h, list(fillers))
        if h % 2 == 1:   # heads 2f, 2f+1 done -> columns f final for block qc
            o_xbars_f(qc, h // 2)
    for g in range(16):
        o_mm_group(3, g)
    ctx.close()


_NC_CACHE = None


def _get_nc():
    global _NC_CACHE
    if _NC_CACHE is None:
        _NC_CACHE = _build_nc()
    return _NC_CACHE


# interleaved head-dim order: row 2i = d_i, row 2i+1 = d_{i+32}
_PHI = np.empty(64, dtype=np.int64)
_PHI[0::2] = np.arange(32)
_PHI[1::2] = np.arange(32) + 32


def _rope_tables_T(pos):
    """Transposed rope tables in the interleaved row order, [128, SEQ]."""
    pos = np.asarray(pos, dtype=np.float32)
    inv = (1.0 / (np.float32(ROPE_THETA)
                  ** (np.arange(0, HEAD_DIM, 2, dtype=np.float32)
                      / np.float32(HEAD_DIM)))).astype(np.float32)  # [32]
    # row r (within 64): dim pair index i = r//2; angle = pos * inv[i]
    ang = inv[(np.arange(64) // 2)][:, None] * pos[None, :]   # [64, SEQ]
    c = np.cos(ang)
    s = np.sin(ang)
    sign = np.where(np.arange(64) % 2 == 0, -1.0, 1.0).astype(np.float32)
    ss = s * sign[:, None]
    cosT = np.concatenate([c, c], axis=0).astype(np.float32)   # [128, SEQ]
    ssT = np.concatenate([ss, ss], axis=0).astype(np.float32)
    return cosT, ssT


def _make_in_maps(input_ids, Wq, Wk, Wv, Wo, position_ids):
    x = np.asarray(input_ids, dtype=np.float32)
    Wq = np.asarray(Wq, dtype=np.float32)
    Wk = np.asarray(Wk, dtype=np.float32)
    Wv = np.asarray(Wv, dtype=np.float32)
    Wo = np.asarray(Wo, dtype=np.float32)
    pos = np.asarray(position_ids)

    maskt = np.triu(np.ones((128, 128), dtype=np.float32)).astype(BF16)

    in_maps = []
    for c in range(N_CORES):
        b, g = c // TP, c % TP
        xTc = np.ascontiguousarray(x[b].T).astype(BF16)
        # q pair-stacked stationaries: pair p = local heads (p, p+4),
        # columns phi-permuted within each head
        qcols = []
        for p in range(4):
            for hh in (p, p + 4):
                base = (g * QH + hh) * HEAD_DIM
                qcols.extend((base + _PHI).tolist())
        wq_t = Wq[:, qcols]                                    # [H, 512]
        # k stacked stationary: kv0 then kv1, phi-permuted
        kcols = []
        for j in range(KVH):
            base = (g * KVH + j) * HEAD_DIM
            kcols.extend((base + _PHI).tolist())
        wk_t = Wk[:, kcols]                                    # [H, 128]
        # v natural
        wv_n = Wv[:, g * KVH * HEAD_DIM:(g + 1) * KVH * HEAD_DIM]
        wall = np.concatenate([wq_t, wk_t, wv_n], axis=1).astype(BF16)
        wo_s = np.ascontiguousarray(
            Wo[g * F_O:(g + 1) * F_O, :]).astype(BF16)
        cosT, ssT = _rope_tables_T(pos[b])
        in_maps.append({
            "xT": np.ascontiguousarray(xTc),
            "wall": np.ascontiguousarray(wall),
            "wo": wo_s,
            "cosT": cosT,
            "ssT": ssT,
            "maskt": maskt,
        })
    return in_maps


def _run(in_maps, trace=False):
    nc = _get_nc()
    kwargs = {}
    if trace:
        _install_profile_hook()
        kwargs["trace"] = True
    return run_bass_kernel_spmd(nc, in_maps, core_ids=list(range(N_CORES)),
                                **kwargs)


def _install_profile_hook():
    """This image's antenv lacks axon_hooks; register the NTFF profile hook
    manually so trace=True yields hardware exec times."""
    if "antenv.axon_hooks" in sys.modules:
        return
    import antenv
    mod = types.ModuleType("antenv.axon_hooks")
    state = {"hook": None}
    mod.set_axon_ntff_profile_hook = lambda h: state.__setitem__("hook", h)
    mod.get_axon_ntff_profile_hook = lambda: state["hook"]
    sys.modules["antenv.axon_hooks"] = mod
    antenv.axon_hooks = mod
    try:
        from trn_agent_boot.trn_boot import _ntff_profile_via_ctypes
        mod.set_axon_ntff_profile_hook(
            _ntff_profile_via_ctypes("/opt/axon/libaxon_pjrt.so"))
    except Exception:
        pass


def kernel(input_ids, Wq, Wk, Wv, Wo, position_ids):
    in_maps = _make_in_maps(input_ids, Wq, Wk, Wv, Wo, position_ids)
    res = _run(in_maps, trace=bool(os.environ.get("KERNEL_TRACE")))
    if os.environ.get("KERNEL_TRACE"):
        print(f"HW exec time: {res.exec_time_ns} ns "
              f"(mean {res.mean_exec_time_ns})")
    out = np.zeros((BATCH, SEQ, HIDDEN), dtype=np.float32)
    for c in range(N_CORES):
        out[c // TP] += res.results[c]["out"]
    return out


# revision 13
# speedup vs baseline: 1.5734x; 1.0871x over previous
"""Trainium2 Bass kernel for a dense-transformer attention block.

Module: y = o_proj(causal_sdpa(rope(q_proj(x)), rope(k_proj(x)), v_proj(x)))
Shapes: x [2, 2048, 2048], 32 q heads / 8 kv heads, head_dim 64, fp32 I/O.

Sharding (8 NeuronCores): 2-way data parallel over batch x 4-way tensor
parallel over heads. Core c handles batch c//4 and head group c%4
(8 q heads, 2 kv heads). Each core produces a partial [2048, 2048]
output (its heads' slice of o_proj); the host sums the 4 partials per
batch.

v3 design (evidence-driven; see NTFF analyses of v1/v2):
- q and k are projected DIRECTLY TRANSPOSED: the projection matmuls use
  W slices as the stationary operand and x^T as the moving operand, so
  qT/kT come out [d, seq] with no transpose instructions at all. Head
  pairs (p, p+4) share one stationary (q head p on partitions 0:64,
  p+4 on 64:128) so each q head sits on the same partition half as its
  kv head (matmul requires equal base partitions). The head-dim rows
  are interleaved (d_i, d_{i+32} adjacent) so the RoPE rotate-half
  partner is one partition away, reachable by DVE stream_shuffle
  (which can only permute within 32-partition quadrants). S is
  invariant to this shared row permutation.
- RoPE runs in the transposed layout: stream_shuffle + 3 tensor ops per
  chunk, reading the projection PSUM and writing bf16 SBUF directly.
  k's RoPE writes straight into kTlo ([kv0 | 0]) and kThi ([0 | kv1]);
  the zero halves are memset once. S matmuls then contract K=128 with
  a half-zero stationary — measured: K=64 matmuls run at half clock
  (HAM stays cold), so zero-padding to full K is the fast path.
- v is projected in natural layout (x-chunk stationary) since the O
  matmul needs v [seq, d] as its moving operand.
- Attention per (qc, h): S^T tiles (k stationary, q moving, fp32 PSUM),
  exp on ACT (scale=0.125, no max subtraction: |0.125 S| < ~10), mask
  by upper-triangle multiply on the diagonal tiles, O accumulated with
  P^T stationary / v moving (measured 35ns/matmul in v1), ones-column
  denominator, then ONE reciprocal + ONE broadcast-multiply evict.
- Emission interleaves: attention block qc carries fillers = the qkv
  chunks of block qc+1 and the o_proj groups of block qc-1, so the ACT
  engine's ~190us of exp overlaps PE work across the whole span.
- o transposes (64 total) go through the DMA XBAR on the sync queue
  (only 68 dispatches live there); output DMA via gpsimd SWDGE.
- PSUM: psB (2 bufs x [128,512]f32; q/k/v/o_proj rotation) = 2 banks,
  psS (2 bufs x [128,2,512]f32) = 4, psO (2 bufs x [128,4,68]f32) = 2.
"""

import os
import sys
import types

import numpy as np

sys.path.insert(0, "/opt/trn_rl_repo")

import concourse.bacc as bacc  # noqa: E402
import concourse.bass as bass  # noqa: E402
import concourse.tile as tile  # noqa: E402
from concourse import mybir  # noqa: E402
from concourse.bass_utils import run_bass_kernel_spmd  # noqa: E402

try:
    import ml_dtypes
    BF16 = ml_dtypes.bfloat16
except ImportError:  # pragma: no cover
    BF16 = np.dtype("bfloat16")

HIDDEN = 2048
SEQ = 2048
BATCH = 2
N_HEADS = 32
N_KV_HEADS = 8
HEAD_DIM = 64
ROPE_THETA = 10000.0

N_CORES = 8
TP = 4                      # head-parallel ways
QH = N_HEADS // TP          # 8 q heads per core
KVH = N_KV_HEADS // TP      # 2 kv heads per core
KT = HIDDEN // 128          # 16 contraction tiles
TT = SEQ // 128             # 16 seq tiles
HD = HEAD_DIM
F_QT = 512                  # 4 pair-stacked qT stationary column blocks
F_KT = 128                  # 1 stacked kT stationary column block
F_V = 128                   # v natural columns (2 kv heads x 64)
F_W = F_QT + F_KT + F_V     # 768
F_O = QH * HEAD_DIM         # 512

FP32 = mybir.dt.float32
BF16_DT = mybir.dt.bfloat16

SHUF_MASK = [i ^ 1 for i in range(32)]


def _build_nc():
    nc = bacc.Bacc("TRN2", target_bir_lowering=False, debug=False)

    xT = nc.dram_tensor("xT", [HIDDEN, SEQ], BF16_DT, kind="ExternalInput")
    wall = nc.dram_tensor("wall", [HIDDEN, F_W], BF16_DT, kind="ExternalInput")
    wo = nc.dram_tensor("wo", [F_O, HIDDEN], BF16_DT, kind="ExternalInput")
    cosT = nc.dram_tensor("cosT", [128, SEQ], FP32, kind="ExternalInput")
    ssT = nc.dram_tensor("ssT", [128, SEQ], FP32, kind="ExternalInput")
    maskt = nc.dram_tensor("maskt", [128, 128], BF16_DT, kind="ExternalInput")
    out = nc.dram_tensor("out", [SEQ, HIDDEN], BF16_DT, kind="ExternalOutput")

    with tile.TileContext(nc) as tc:
        _emit(nc, tc, xT, wall, wo, cosT, ssT, maskt, out)
    nc.compile()
    return nc


def _bcast_last(ap, n):
    """Append a step-0 (broadcast) dim of size n to an AP."""
    return bass.AP(tensor=ap.tensor, offset=ap.offset, ap=list(ap.ap) + [[0, n]])


def _emit(nc, tc, xT, wall, wo, cosT, ssT, maskt, out):
    from contextlib import ExitStack
    ctx = ExitStack()
    Exp = mybir.ActivationFunctionType.Exp
    mult = mybir.AluOpType.mult

    const = ctx.enter_context(tc.tile_pool(name="const", bufs=1))
    persist = ctx.enter_context(tc.tile_pool(name="persist", bufs=1))

    # ---- weights/tables on the scalar HWDGE queue ----
    wk = [const.tile([128, F_W], BF16_DT, name=f"wk{k}") for k in range(KT)]
    w_r = wall[:].rearrange("(k p) f -> p k f", p=128)
    for k in range(KT):
        nc.scalar.dma_start(out=wk[k][:], in_=w_r[:, k, :])
    cosT2 = const.tile([128, SEQ], FP32)
    ssT2 = const.tile([128, SEQ], FP32)
    nc.scalar.dma_start(out=cosT2[:], in_=cosT[:])
    nc.scalar.dma_start(out=ssT2[:], in_=ssT[:])
    mask_sb = const.tile([128, 128], BF16_DT)
    nc.scalar.dma_start(out=mask_sb[:], in_=maskt[:])
    wo_sb = const.tile([128, F_O // 128, HIDDEN], BF16_DT)
    nc.scalar.dma_start(out=wo_sb[:], in_=wo[:].rearrange("(k p) d -> p k d", p=128))

    # ---- x as 4 per-t-block tensors on the sync HWDGE queue ----
    xb = [const.tile([128, KT, 512], BF16_DT, name=f"xb{b}") for b in range(4)]
    xT_r = xT[:].rearrange("(k p) t -> p k t", p=128)
    for b in range(4):
        nc.sync.dma_start(out=xb[b][:], in_=xT_r[:, :, bass.ts(b, 512)])

    # ---- persistent SBUF tensors ----
    qT = [persist.tile([128, SEQ], BF16_DT, name=f"qT{p}") for p in range(4)]
    kTlo = persist.tile([128, SEQ], BF16_DT, name="kTlo")   # [kv0 | zeros]
    kThi = persist.tile([128, SEQ], BF16_DT, name="kThi")   # [zeros | kv1]
    v_sb = [persist.tile([128, 4, KVH, HD + 1], BF16_DT, name=f"v{b}")
            for b in range(4)]
    o_sb = [persist.tile([128, 4, F_O], BF16_DT, name=f"ob{qc}") for qc in range(4)]
    # 3-parity oT buffers: o_mm(qc) reads oT[qc % 3] while incremental
    # XBARs for later blocks write the other parities.
    oT_sb = [[persist.tile([128, 512], BF16_DT, name=f"oT{par}_{f}")
              for f in range(4)] for par in range(3)]
    nc.gpsimd.memset(kTlo[64:128, :], 0.0)
    nc.gpsimd.memset(kThi[0:64, :], 0.0)
    for b in range(4):
        nc.gpsimd.memset(v_sb[b][:, :, :, HD:HD + 1], 1.0)

    # ---- pools ----
    psB = ctx.enter_context(tc.tile_pool(name="psB", bufs=2, space="PSUM"))
    psS = ctx.enter_context(tc.tile_pool(name="psS", bufs=2, space="PSUM"))
    psO = ctx.enter_context(tc.tile_pool(name="psO", bufs=2, space="PSUM"))
    bwork = ctx.enter_context(tc.tile_pool(name="bwork", bufs=2))
    att = ctx.enter_context(tc.tile_pool(name="att", bufs=6))
    fwork = ctx.enter_context(tc.tile_pool(name="fwork", bufs=3))

    def q_chunk(b, p):
        """Pair-stacked transposed q projection: heads (p, p+4), 512 s-cols."""
        bcols = bass.ds(b * 512, 512)
        ps = psB.tile([128, 512], FP32, tag="pb", name=f"q{b}_{p}")
        for k in range(KT):
            nc.tensor.matmul(ps[:], wk[k][:, bass.ts(p, 128)], xb[b][:, k, :],
                             start=(k == 0), stop=(k == KT - 1))
        shf = bwork.tile([128, 512], FP32, tag="shf")
        tm = bwork.tile([128, 512], BF16_DT, tag="tm")
        dst = qT[p][:, bcols]
        # ps-reading ops first so the PSUM slot frees as early as possible
        nc.vector.stream_shuffle(shf[:], ps[:], mask=SHUF_MASK)
        nc.vector.tensor_tensor(dst, ps[:], cosT2[:, bcols], op=mult)
        nc.vector.tensor_tensor(tm[:], shf[:], ssT2[:, bcols], op=mult)
        nc.vector.tensor_add(dst, dst, tm[:])

    def k_chunk(b):
        """Stacked transposed k projection -> kTlo[0:64], kThi[64:128]."""
        bcols = bass.ds(b * 512, 512)
        ps = psB.tile([128, 512], FP32, tag="pb", name=f"k{b}")
        for k in range(KT):
            nc.tensor.matmul(ps[:], wk[k][:, F_QT:F_QT + 128], xb[b][:, k, :],
                             start=(k == 0), stop=(k == KT - 1))
        shf = bwork.tile([128, 512], FP32, tag="shf")
        tm = bwork.tile([128, 512], BF16_DT, tag="tm")
        nc.vector.stream_shuffle(shf[:], ps[:], mask=SHUF_MASK)
        for lo, hi, kt in ((0, 64, kTlo), (64, 128, kThi)):
            nc.vector.tensor_tensor(kt[lo:hi, bcols], ps[lo:hi, :],
                                    cosT2[lo:hi, bcols], op=mult)
        nc.vector.tensor_tensor(tm[:], shf[:], ssT2[:, bcols], op=mult)
        for lo, hi, kt in ((0, 64, kTlo), (64, 128, kThi)):
            dst = kt[lo:hi, bcols]
            nc.vector.tensor_add(dst, dst, tm[lo:hi, :])

    def v_block(b):
        """Natural-layout v projection for the 4 seq tiles of block b."""
        ps = psB.tile([128, 512], FP32, tag="pb", name=f"v{b}")
        pv = ps[:].rearrange("p (j f) -> p j f", f=128)
        for j in range(4):
            for k in range(KT):
                nc.tensor.matmul(pv[:, j, :],
                                 xb[b][:, k, bass.ts(j, 128)],
                                 wk[k][:, F_QT + F_KT:F_W],
                                 start=(j == 0 and k == 0), stop=(k == KT - 1),
                                 skip_group_check=(j > 0))
        nc.vector.tensor_copy(
            v_sb[b][:, :, :, 0:HD],
            pv.rearrange("p j (h d) -> p j h d", d=HD))

    # PSUM accumulation-group notes: all four O accumulators share one
    # bank (4x68 fp32). start=True clears the whole bank's has_written
    # state, so ONLY the bank's first matmul (ik=0, j=0) carries
    # start=True; siblings rely on that clear (same-engine order).
    def attn_head(qc, h, fillers):
        tj = [4 * qc + j for j in range(4)]
        n_ik = tj[3] + 1
        jv = h // 4
        kTv = kTlo if h < 4 else kThi
        qTp = qT[h % 4]
        hsl = bass.ds(h * HD, HD)
        Oall = psO.tile([128, 4, 68], FP32, tag="O", name=f"Op{h}_{qc}")
        Ops = [Oall[:, j, 0:HD + 1] for j in range(4)]
        for g in range(0, n_ik, 2):
            members = [ik for ik in (g, g + 1) if ik < n_ik]
            j0s = [max(0, ik - 4 * qc) for ik in members]
            stp = psS.tile([128, 2, 512], FP32, tag="st")
            p_sb = att.tile([128, 2, 512], BF16_DT, tag="p")
            for m, ik in enumerate(members):
                j0 = j0s[m]
                nc.tensor.matmul(
                    stp[:, m, bass.ds(j0 * 128, 512 - j0 * 128)],
                    kTv[:, bass.ts(ik, 128)],
                    qTp[:, bass.ds(qc * 512 + j0 * 128, 512 - j0 * 128)],
                    start=True, stop=True)
            # one exp over both key tiles, from the leftmost live column on
            jmin = min(j0s)
            ecols = bass.ds(jmin * 128, 512 - jmin * 128)
            nc.scalar.activation(p_sb[:, 0:len(members), ecols],
                                 stp[:, 0:len(members), ecols],
                                 Exp, scale=0.125)
            if g >= 4 * qc:
                # both members are diagonal tiles: one strided DVE op masks
                # member 0 at column j0*128 and member 1 at (j0+1)*128
                # (offsets differ by 512+128=640 -> a regular 2-level AP).
                j0 = j0s[0]
                base = p_sb[:, 0, bass.ds(j0 * 128, 128)]
                pair = bass.AP(tensor=base.tensor, offset=base.offset,
                               ap=[base.ap[0], [640, 2]] + list(base.ap[1:]))
                mb = bass.AP(tensor=mask_sb[:].tensor, offset=mask_sb[:].offset,
                             ap=[mask_sb[:].ap[0], [0, 2]]
                             + list(mask_sb[:].ap[1:]))
                nc.vector.tensor_tensor(pair, pair, mb, op=mult)
            for m, ik in enumerate(members):
                j0 = j0s[m]
                for j in range(j0, 4):
                    nc.tensor.matmul(
                        Ops[j], p_sb[:, m, bass.ts(j, 128)],
                        v_sb[ik // 4][:, ik % 4, jv, :],
                        start=(ik == 0 and j == 0),
                        stop=(ik == tj[j]),
                        skip_group_check=(j > 0))
            if fillers:
                fillers.pop(0)()
        # normalization: one reciprocal + one bcast-multiply for all 4 tiles
        rc4 = fwork.tile([128, 4], FP32, tag="rc")
        nc.vector.reciprocal(rc4[:].unsqueeze(2), Oall[:, :, HD:HD + 1])
        nc.vector.tensor_tensor(o_sb[qc][:, :, hsl], Oall[:, :, 0:HD],
                                _bcast_last(rc4[:], HD), op=mult)
        for f in fillers:
            f()

    def o_xbars_f(qc, f):
        """XBAR-transpose feature columns f of block qc (needs heads 2f,2f+1)."""
        for j in range(4):
            nc.sync.dma_start_transpose(oT_sb[qc % 3][f][:, bass.ts(j, 128)],
                                        o_sb[qc][:, j, bass.ts(f, 128)])

    def o_mm_group(qc, gi):
        j, nch = divmod(gi, 4)
        t = 4 * qc + j
        po = psB.tile([128, 512], FP32, tag="pb", name=f"po{qc}_{gi}")
        for kf in range(4):
            nc.tensor.matmul(po[:], oT_sb[qc % 3][kf][:, bass.ts(j, 128)],
                             wo_sb[:, kf, bass.ts(nch, 512)],
                             start=(kf == 0), stop=(kf == 3))
        ost = fwork.tile([128, 512], BF16_DT, tag="ost")
        # eviction engine: ACT in ACT-light phases (blocks 0 and the tail
        # half of block 3) to keep the DVE FIFO short; DMA dispatch split
        # between the gpsimd SWDGE and the (tail-idle) sync queue.
        use_act = qc == 0 or (qc == 3 and gi % 2 == 0)
        if use_act:
            nc.scalar.copy(ost[:], po[:])
        else:
            nc.vector.tensor_copy(ost[:], po[:])
        dma_eng = nc.sync if (qc == 3 and gi % 2 == 1) else nc.gpsimd
        dma_eng.dma_start(out=out[bass.ts(t, 128), bass.ts(nch, 512)],
                          in_=ost[:])

    # ---- emission ----
    # Attention item order: blocks 0, 1 whole; then qc2 h0-3; then qc3 heads
    # zipped with qc2 h4-7 (balances the exp-heavy late blocks against PE
    # work); then qc3 h4-7. Fillers (qkv chunks of the next block, o_proj
    # groups of finished blocks) are attached per item, qkv earliest.
    def om(qc, g):
        return lambda: o_mm_group(qc, g)

    qk = {b: ([lambda b=b, p=p: q_chunk(b, p) for p in range(4)]
              + [lambda b=b: k_chunk(b), lambda b=b: v_block(b)])
          for b in range(1, 4)}

    items = []   # (qc, h, fillers)
    items += [(0, 0, qk[1][0:2]), (0, 1, qk[1][2:4]), (0, 2, [qk[1][4]]),
              (0, 3, [qk[1][5]]), (0, 4, []), (0, 5, []), (0, 6, []),
              (0, 7, [])]
    items += [(1, 0, qk[2][0:2]), (1, 1, qk[2][2:4]), (1, 2, qk[2][4:6]),
              (1, 3, [om(0, g) for g in range(0, 3)]),
              (1, 4, [om(0, g) for g in range(3, 6)]),
              (1, 5, [om(0, g) for g in range(6, 9)]),
              (1, 6, [om(0, g) for g in range(9, 13)]),
              (1, 7, [om(0, g) for g in range(13, 16)])]
    items += [(2, 0, qk[3][4:6]), (2, 1, qk[3][0:2]), (2, 2, qk[3][2:4]),
              (2, 3, [om(1, g) for g in range(0, 2)])]
    items += [(3, 0, []), (2, 4, [om(1, g) for g in range(2, 6)]),
              (3, 1, []), (2, 5, [om(1, g) for g in range(6, 10)]),
              (3, 2, []), (2, 6, [om(1, g) for g in range(10, 13)]),
              (3, 3, []), (2, 7, [om(1, g) for g in range(13, 16)])]
    items += [(3, 4, [om(2, g) for g in range(0, 4)]),
              (3, 5, [om(2, g) for g in range(4, 8)]),
              (3, 6, [om(2, g) for g in range(8, 12)]),
              (3, 7, [om(2, g) for g in range(12, 16)])]

    q_chunk(0, 0)
    q_chunk(0, 1)
    k_chunk(0)
    q_chunk(0, 2)
    q_chunk(0, 3)
    v_block(0)
    for qc, h, fillers in items:
        attn_head(qc, h, list(fillers))
        if h % 2 == 1:   # heads 2f, 2f+1 done -> columns f final for block qc
            o_xbars_f(qc, h // 2)
    for g in range(16):
        o_mm_group(3, g)
    ctx.close()


_NC_CACHE = None


def _get_nc():
    global _NC_CACHE
    if _NC_CACHE is None:
        _NC_CACHE = _build_nc()
    return _NC_CACHE


# interleaved head-dim order: row 2i = d_i, row 2i+1 = d_{i+32}
_PHI = np.empty(64, dtype=np.int64)
_PHI[0::2] = np.arange(32)
_PHI[1::2] = np.arange(32) + 32


def _rope_tables_T(pos):
    """Transposed rope tables in the interleaved row order, [128, SEQ]."""
    pos = np.asarray(pos, dtype=np.float32)
    inv = (1.0 / (np.float32(ROPE_THETA)
                  ** (np.arange(0, HEAD_DIM, 2, dtype=np.float32)
                      / np.float32(HEAD_DIM)))).astype(np.float32)  # [32]
    # row r (within 64): dim pair index i = r//2; angle = pos * inv[i]
    ang = inv[(np.arange(64) // 2)][:, None] * pos[None, :]   # [64, SEQ]
    c = np.cos(ang)
    s = np.sin(ang)
    sign = np.where(np.arange(64) % 2 == 0, -1.0, 1.0).astype(np.float32)
    ss = s * sign[:, None]
    cosT = np.concatenate([c, c], axis=0).astype(np.float32)   # [128, SEQ]
    ssT = np.concatenate([ss, ss], axis=0).astype(np.float32)
    return cosT, ssT


def _make_in_maps(input_ids, Wq, Wk, Wv, Wo, position_ids):
    x = np.asarray(input_ids, dtype=np.float32)
    Wq = np.asarray(Wq, dtype=np.float32)
    Wk = np.asarray(Wk, dtype=np.float32)
    Wv = np.asarray(Wv, dtype=np.float32)
    Wo = np.asarray(Wo, dtype=np.float32)
    pos = np.asarray(position_ids)

    maskt = np.triu(np.ones((128, 128), dtype=np.float32)).astype(BF16)

    in_maps = []
    for c in range(N_CORES):
        b, g = c // TP, c % TP
        xTc = np.ascontiguousarray(x[b].T).astype(BF16)
        # q pair-stacked stationaries: pair p = local heads (p, p+4),
        # columns phi-permuted within each head
        qcols = []
        for p in range(4):
            for hh in (p, p + 4):
                base = (g * QH + hh) * HEAD_DIM
                qcols.extend((base + _PHI).tolist())
        wq_t = Wq[:, qcols]                                    # [H, 512]
        # k stacked stationary: kv0 then kv1, phi-permuted
        kcols = []
        for j in range(KVH):
            base = (g * KVH + j) * HEAD_DIM
            kcols.extend((base + _PHI).tolist())
        wk_t = Wk[:, kcols]                                    # [H, 128]
        # v natural
        wv_n = Wv[:, g * KVH * HEAD_DIM:(g + 1) * KVH * HEAD_DIM]
        wall = np.concatenate([wq_t, wk_t, wv_n], axis=1).astype(BF16)
        wo_s = np.ascontiguousarray(
            Wo[g * F_O:(g + 1) * F_O, :]).astype(BF16)
        cosT, ssT = _rope_tables_T(pos[b])
        in_maps.append({
            "xT": np.ascontiguousarray(xTc),
            "wall": np.ascontiguousarray(wall),
            "wo": wo_s,
            "cosT": cosT,
            "ssT": ssT,
            "maskt": maskt,
        })
    return in_maps


def _run(in_maps, trace=False):
    nc = _get_nc()
    kwargs = {}
    if trace:
        _install_profile_hook()
        kwargs["trace"] = True
    return run_bass_kernel_spmd(nc, in_maps, core_ids=list(range(N_CORES)),
                                **kwargs)


def _install_profile_hook():
    """This image's antenv lacks axon_hooks; register the NTFF profile hook
    manually so trace=True yields hardware exec times."""
    if "antenv.axon_hooks" in sys.modules:
        return
    import antenv
    mod = types.ModuleType("antenv.axon_hooks")
    state = {"hook": None}
    mod.set_axon_ntff_profile_hook = lambda h: state.__setitem__("hook", h)
    mod.get_axon_ntff_profile_hook = lambda: state["hook"]
    sys.modules["antenv.axon_hooks"] = mod
    antenv.axon_hooks = mod
    try:
        from trn_agent_boot.trn_boot import _ntff_profile_via_ctypes
        mod.set_axon_ntff_profile_hook(
            _ntff_profile_via_ctypes("/opt/axon/libaxon_pjrt.so"))
    except Exception:
        pass


def kernel(input_ids, Wq, Wk, Wv, Wo, position_ids):
    in_maps = _make_in_maps(input_ids, Wq, Wk, Wv, Wo, position_ids)
    res = _run(in_maps, trace=bool(os.environ.get("KERNEL_TRACE")))
    if os.environ.get("KERNEL_TRACE"):
        print(f"HW exec time: {res.exec_time_ns} ns "
              f"(mean {res.mean_exec_time_ns})")
    out = np.zeros((BATCH, SEQ, HIDDEN), dtype=np.float32)
    for c in range(N_CORES):
        out[c // TP] += res.results[c]["out"]
    return out


# revision 28
# speedup vs baseline: 1.6951x; 1.0773x over previous
"""Trainium2 Bass kernel for a dense-transformer attention block.

Module: y = o_proj(causal_sdpa(rope(q_proj(x)), rope(k_proj(x)), v_proj(x)))
Shapes: x [2, 2048, 2048], 32 q heads / 8 kv heads, head_dim 64, fp32 I/O.

Sharding (8 NeuronCores): 2-way data parallel over batch x 4-way tensor
parallel over heads. Core c handles batch c//4 and head group c%4
(8 q heads, 2 kv heads). Each core produces a partial [2048, 2048]
output (its heads' slice of o_proj); the host sums the 4 partials per
batch.

v3 design (evidence-driven; see NTFF analyses of v1/v2):
- q and k are projected DIRECTLY TRANSPOSED: the projection matmuls use
  W slices as the stationary operand and x^T as the moving operand, so
  qT/kT come out [d, seq] with no transpose instructions at all. Head
  pairs (p, p+4) share one stationary (q head p on partitions 0:64,
  p+4 on 64:128) so each q head sits on the same partition half as its
  kv head (matmul requires equal base partitions). The head-dim rows
  are interleaved (d_i, d_{i+32} adjacent) so the RoPE rotate-half
  partner is one partition away, reachable by DVE stream_shuffle
  (which can only permute within 32-partition quadrants). S is
  invariant to this shared row permutation.
- RoPE runs in the transposed layout: stream_shuffle + 3 tensor ops per
  chunk, reading the projection PSUM and writing bf16 SBUF directly.
  k's RoPE writes straight into kTlo ([kv0 | 0]) and kThi ([0 | kv1]);
  the zero halves are memset once. S matmuls then contract K=128 with
  a half-zero stationary — measured: K=64 matmuls run at half clock
  (HAM stays cold), so zero-padding to full K is the fast path.
- v is projected in natural layout (x-chunk stationary) since the O
  matmul needs v [seq, d] as its moving operand.
- Attention per (qc, h): S^T tiles (k stationary, q moving, fp32 PSUM),
  exp on ACT (scale=0.125, no max subtraction: |0.125 S| < ~10), mask
  by upper-triangle multiply on the diagonal tiles, O accumulated with
  P^T stationary / v moving (measured 35ns/matmul in v1), ones-column
  denominator, then ONE reciprocal + ONE broadcast-multiply evict.
- Emission interleaves: attention block qc carries fillers = the qkv
  chunks of block qc+1 and the o_proj groups of block qc-1, so the ACT
  engine's ~190us of exp overlaps PE work across the whole span.
- o transposes (64 total) go through the DMA XBAR on the sync queue
  (only 68 dispatches live there); output DMA via gpsimd SWDGE.
- PSUM: psB (2 bufs x [128,512]f32; q/k/v/o_proj rotation) = 2 banks,
  psS (2 bufs x [128,2,512]f32) = 4, psO (2 bufs x [128,4,68]f32) = 2.
"""

import os
import sys
import types

import numpy as np

sys.path.insert(0, "/opt/trn_rl_repo")

import concourse.bacc as bacc  # noqa: E402
import concourse.bass as bass  # noqa: E402
import concourse.tile as tile  # noqa: E402
from concourse import mybir  # noqa: E402
from concourse.bass_utils import run_bass_kernel_spmd  # noqa: E402

try:
    import ml_dtypes
    BF16 = ml_dtypes.bfloat16
except ImportError:  # pragma: no cover
    BF16 = np.dtype("bfloat16")

HIDDEN = 2048
SEQ = 2048
BATCH = 2
N_HEADS = 32
N_KV_HEADS = 8
HEAD_DIM = 64
ROPE_THETA = 10000.0

N_CORES = 8
TP = 4                      # head-parallel ways
QH = N_HEADS // TP          # 8 q heads per core
KVH = N_KV_HEADS // TP      # 2 kv heads per core
KT = HIDDEN // 128          # 16 contraction tiles
TT = SEQ // 128             # 16 seq tiles
HD = HEAD_DIM
F_QT = 512                  # 4 pair-stacked qT stationary column blocks
F_KT = 128                  # 1 stacked kT stationary column block
F_V = 128                   # v natural columns (2 kv heads x 64)
F_W = F_QT + F_KT + F_V     # 768
F_O = QH * HEAD_DIM         # 512

FP32 = mybir.dt.float32
BF16_DT = mybir.dt.bfloat16

SHUF_MASK = [i ^ 1 for i in range(32)]


def _build_nc():
    nc = bacc.Bacc("TRN2", target_bir_lowering=False, debug=False)

    xT = nc.dram_tensor("xT", [HIDDEN, SEQ], BF16_DT, kind="ExternalInput")
    wall = nc.dram_tensor("wall", [HIDDEN, F_W], BF16_DT, kind="ExternalInput")
    wo = nc.dram_tensor("wo", [F_O, HIDDEN], BF16_DT, kind="ExternalInput")
    cosT = nc.dram_tensor("cosT", [128, SEQ], FP32, kind="ExternalInput")
    ssT = nc.dram_tensor("ssT", [128, SEQ], FP32, kind="ExternalInput")
    maskt = nc.dram_tensor("maskt", [128, 128], BF16_DT, kind="ExternalInput")
    out = nc.dram_tensor("out", [SEQ, HIDDEN], BF16_DT, kind="ExternalOutput")

    with tile.TileContext(nc) as tc:
        _emit(nc, tc, xT, wall, wo, cosT, ssT, maskt, out)
    nc.compile()
    return nc


def _bcast(ap, n, axis_pos=1):
    """Insert a step-0 (broadcast) dim of size n into an AP at axis_pos."""
    new_ap = list(ap.ap)
    new_ap.insert(axis_pos, [0, n])
    return bass.AP(tensor=ap.tensor, offset=ap.offset, ap=new_ap)


def _bcast_last(ap, n):
    """Append a step-0 (broadcast) dim of size n to an AP."""
    return bass.AP(tensor=ap.tensor, offset=ap.offset, ap=list(ap.ap) + [[0, n]])


def _emit(nc, tc, xT, wall, wo, cosT, ssT, maskt, out):
    from contextlib import ExitStack
    ctx = ExitStack()
    Exp = mybir.ActivationFunctionType.Exp
    mult = mybir.AluOpType.mult

    const = ctx.enter_context(tc.tile_pool(name="const", bufs=1))
    persist = ctx.enter_context(tc.tile_pool(name="persist", bufs=1))

    # ---- weights/tables on the scalar HWDGE queue ----
    wk = [const.tile([128, F_W], BF16_DT, name=f"wk{k}") for k in range(KT)]
    w_r = wall[:].rearrange("(k p) f -> p k f", p=128)
    for k in range(KT):
        nc.scalar.dma_start(out=wk[k][:], in_=w_r[:, k, :])
    cosT2 = const.tile([128, SEQ], FP32)
    ssT2 = const.tile([128, SEQ], FP32)
    nc.scalar.dma_start(out=cosT2[:], in_=cosT[:])
    nc.scalar.dma_start(out=ssT2[:], in_=ssT[:])
    mask_sb = const.tile([128, 128], BF16_DT)
    nc.scalar.dma_start(out=mask_sb[:], in_=maskt[:])
    wo_sb = const.tile([128, F_O // 128, HIDDEN], BF16_DT)

    # ---- x as 8 per-half-t-block tensors on the sync HWDGE queue (finer
    # arrival granularity: the first projection chain starts ~4us earlier) ----
    xbh = [[const.tile([128, KT // 2, 512], BF16_DT, name=f"xb{b}_{hh}")
            for hh in range(2)] for b in range(4)]
    xT_r = xT[:].rearrange("(k p) t -> p k t", p=128)

    def x_dmas(bs):
        for b in bs:
            for hh in range(2):
                nc.sync.dma_start(
                    out=xbh[b][hh][:],
                    in_=xT_r[:, bass.ds(hh * (KT // 2), KT // 2),
                             bass.ts(b, 512)])

    # only block 0 up front: emitting the rest after the prologue keeps the
    # first projection chain's (hoisted) DMA wait at just xbh[0]
    x_dmas([0])

    def xk(b, k):
        return xbh[b][k // (KT // 2)][:, k % (KT // 2), :]

    # ---- persistent SBUF tensors ----
    qT = [persist.tile([128, SEQ], BF16_DT, name=f"qT{p}") for p in range(4)]
    kTlo = persist.tile([128, SEQ], BF16_DT, name="kTlo")   # [kv0 | zeros]
    kThi = persist.tile([128, SEQ], BF16_DT, name="kThi")   # [zeros | kv1]
    # stride 72 (not 65): keeps every XBAR destination offset 16B-aligned
    v_sb = [persist.tile([128, 4, KVH, 72], BF16_DT, name=f"v{b}")
            for b in range(4)]
    o_sb = [persist.tile([128, 4, F_O], BF16_DT, name=f"ob{qc}") for qc in range(4)]
    # 3-parity oT buffers: o_mm(qc) reads oT[qc % 3] while incremental
    # XBARs for later blocks write the other parities.
    oT_sb = [[persist.tile([128, 512], BF16_DT, name=f"oT{par}_{f}")
              for f in range(4)] for par in range(3)]
    nc.gpsimd.memset(kTlo[64:128, :], 0.0)
    nc.gpsimd.memset(kThi[0:64, :], 0.0)
    for b in range(4):
        nc.gpsimd.memset(v_sb[b][:, :, :, HD:HD + 1], 1.0)

    # ---- pools ----
    psB = ctx.enter_context(tc.tile_pool(name="psB", bufs=2, space="PSUM"))
    psS = ctx.enter_context(tc.tile_pool(name="psS", bufs=2, space="PSUM"))
    psO = ctx.enter_context(tc.tile_pool(name="psO", bufs=2, space="PSUM"))
    bwork = ctx.enter_context(tc.tile_pool(name="bwork", bufs=2))
    att = ctx.enter_context(tc.tile_pool(name="att", bufs=6))
    fwork = ctx.enter_context(tc.tile_pool(name="fwork", bufs=3))

    def q_chunk(b, p):
        """Pair-stacked transposed q projection: heads (p, p+4), 512 s-cols."""
        bcols = bass.ds(b * 512, 512)
        ps = psB.tile([128, 512], FP32, tag="pb", name=f"q{b}_{p}")
        for k in range(KT):
            nc.tensor.matmul(ps[:], wk[k][:, bass.ts(p, 128)], xb[b][:, k, :],
                             start=(k == 0), stop=(k == KT - 1))
        shf = bwork.tile([128, 512], FP32, tag="shf")
        tm = bwork.tile([128, 512], BF16_DT, tag="tm")
        dst = qT[p][:, bcols]
        # ps-reading ops first so the PSUM slot frees as early as possible
        nc.vector.stream_shuffle(shf[:], ps[:], mask=SHUF_MASK)
        nc.vector.tensor_tensor(dst, ps[:], cosT2[:, bcols], op=mult)
        nc.vector.tensor_tensor(tm[:], shf[:], ssT2[:, bcols], op=mult)
        nc.vector.tensor_add(dst, dst, tm[:])

    def k_chunk(b):
        """Stacked transposed k projection -> kTlo[0:64], kThi[64:128]."""
        bcols = bass.ds(b * 512, 512)
        ps = psB.tile([128, 512], FP32, tag="pb", name=f"k{b}")
        for k in range(KT):
            nc.tensor.matmul(ps[:], wk[k][:, F_QT:F_QT + 128], xb[b][:, k, :],
                             start=(k == 0), stop=(k == KT - 1))
        shf = bwork.tile([128, 512], FP32, tag="shf")
        tm = bwork.tile([128, 512], BF16_DT, tag="tm")
        nc.vector.stream_shuffle(shf[:], ps[:], mask=SHUF_MASK)
        for lo, hi, kt in ((0, 64, kTlo), (64, 128, kThi)):
            nc.vector.tensor_tensor(kt[lo:hi, bcols], ps[lo:hi, :],
                                    cosT2[lo:hi, bcols], op=mult)
        nc.vector.tensor_tensor(tm[:], shf[:], ssT2[:, bcols], op=mult)
        for lo, hi, kt in ((0, 64, kTlo), (64, 128, kThi)):
            dst = kt[lo:hi, bcols]
            nc.vector.tensor_add(dst, dst, tm[lo:hi, :])

    def v_block(b):
        """Natural-layout v projection for the 4 seq tiles of block b."""
        ps = psB.tile([128, 512], FP32, tag="pb", name=f"v{b}")
        pv = ps[:].rearrange("p (j f) -> p j f", f=128)
        for j in range(4):
            for k in range(KT):
                nc.tensor.matmul(pv[:, j, :],
                                 xb[b][:, k, bass.ts(j, 128)],
                                 wk[k][:, F_QT + F_KT:F_W],
                                 start=(j == 0 and k == 0), stop=(k == KT - 1),
                                 skip_group_check=(j > 0))
        nc.vector.tensor_copy(
            v_sb[b][:, :, :, 0:HD],
            pv.rearrange("p j (h d) -> p j h d", d=HD))

    # Attention processes HEAD PAIRS (hp, hp+4): the pair shares one qT
    # tile (hp on partitions 0:64, hp+4 on 64:128), and the two S matmuls
    # per key tile are K=64 row-split (kTlo rows 0:64 / kThi rows 64:128,
    # tile_position auto-derived) so they execute CONCURRENTLY on disjoint
    # array halves — measured ~224ns for the pair of 512-col matmuls, 2x
    # the serial K=128 rate, with HAM staying warm (both halves busy).
    # PSUM accumulation-group notes: each head's four O accumulators share
    # one bank (4x68 fp32); only the bank's first matmul (ik=0, j=0)
    # carries start=True, siblings rely on that clear (same-engine order).
    def attn_pair(qc, hp, fillers):
        tj = [4 * qc + j for j in range(4)]
        n_ik = tj[3] + 1
        qTp = qT[hp]
        Oalls = [psO.tile([128, 4, 68], FP32, tag="O", name=f"Op{qc}_{hp}_{m}")
                 for m in range(2)]
        for ik in range(n_ik):
            j0 = max(0, ik - 4 * qc)
            cols = bass.ds(j0 * 128, 512 - j0 * 128)
            qcols = bass.ds(qc * 512 + j0 * 128, 512 - j0 * 128)
            ksl = bass.ts(ik, 128)
            stp = psS.tile([128, 2, 512], FP32, tag="st")
            p_sb = att.tile([128, 2, 512], BF16_DT, tag="p")
            nc.tensor.matmul(stp[:, 0, cols], kTlo[0:64, ksl],
                             qTp[0:64, qcols], start=True, stop=True)
            nc.tensor.matmul(stp[:, 1, cols], kThi[64:128, ksl],
                             qTp[64:128, qcols], start=True, stop=True)
            nc.scalar.activation(p_sb[:, :, cols], stp[:, :, cols],
                                 Exp, scale=0.125)
            # filler PE work lands between the S matmuls and the O matmuls
            # so the exp latency hides behind it instead of stalling the
            # in-order PE stream
            if fillers:
                fillers.pop(0)()
            if ik >= 4 * qc:   # diagonal: mask sub-tile j0 of both heads
                nc.vector.tensor_mul(
                    p_sb[:, :, bass.ts(j0, 128)],
                    p_sb[:, :, bass.ts(j0, 128)],
                    _bcast(mask_sb[:], 2))
            for m in range(2):
                for j in range(j0, 4):
                    nc.tensor.matmul(
                        Oalls[m][:, j, 0:HD + 1], p_sb[:, m, bass.ts(j, 128)],
                        v_sb[ik // 4][:, ik % 4, m, 0:HD + 1],
                        start=(ik == 0 and j == 0),
                        stop=(ik == tj[j]),
                        skip_group_check=(j > 0))
        # normalization: one reciprocal + one bcast-multiply per head
        for m in range(2):
            h = hp + 4 * m
            rc4 = fwork.tile([128, 4], FP32, tag="rc")
            nc.vector.reciprocal(rc4[:].unsqueeze(2), Oalls[m][:, :, HD:HD + 1])
            nc.vector.tensor_tensor(o_sb[qc][:, :, bass.ds(h * HD, HD)],
                                    Oalls[m][:, :, 0:HD],
                                    _bcast_last(rc4[:], HD), op=mult)
        for f in fillers:
            f()

    def o_xbars_f(qc, f):
        """XBAR-transpose feature columns f of block qc (needs heads 2f,2f+1).
        Block 3's last transposes split across sync+scalar (both idle then)
        to halve the ~1.2us/dispatch serialization in the tail."""
        for j in range(4):
            eng = nc.scalar if (qc == 3 and f >= 1 and j % 2 == 1) else nc.sync
            eng.dma_start_transpose(oT_sb[qc % 3][f][:, bass.ts(j, 128)],
                                    o_sb[qc][:, j, bass.ts(f, 128)])

    def o_mm_group(qc, gi):
        j, nch = divmod(gi, 4)
        t = 4 * qc + j
        po = psB.tile([128, 512], FP32, tag="pb", name=f"po{qc}_{gi}")
        for i, kf in enumerate((0, 2, 1, 3)):
            nc.tensor.matmul(po[:], oT_sb[qc % 3][kf][:, bass.ts(j, 128)],
                             wo_sb[:, kf, bass.ts(nch, 512)],
                             start=(i == 0), stop=(i == 3))
        ost = fwork.tile([128, 512], BF16_DT, tag="ost")
        # eviction engine: ACT in ACT-light phases (blocks 0 and the tail
        # half of block 3) to keep the DVE FIFO short; DMA dispatch split
        # between the gpsimd SWDGE and the (tail-idle) sync queue.
        use_act = qc == 3 and gi % 2 == 0
        if use_act:
            nc.scalar.copy(ost[:], po[:])
        else:
            nc.vector.tensor_copy(ost[:], po[:])
        if qc == 3:
            dma_eng = nc.sync if gi % 2 == 0 else nc.gpsimd
        else:
            dma_eng = nc.sync if qc == 2 else nc.gpsimd
        dma_eng.dma_start(out=out[bass.ts(t, 128), bass.ts(nch, 512)],
                          in_=ost[:])

    # ---- emission ----
    # Attention item order: blocks 0, 1 whole (4 pairs each); qc2 pairs 0-1;
    # then qc3 pairs zipped with qc2 pairs 2-3; qc3 pairs 2-3 last. Fillers
    # (qkv chunks of later blocks, o_proj groups of finished blocks) pop one
    # per key-tile iteration inside each pair. XBARs for feature block f of
    # a block fire once the pairs holding heads 2f,2f+1 have evicted:
    # f0+f2 after pair 1, f1+f3 after pair 3.
    def om(qc, g):
        return lambda: o_mm_group(qc, g)

    qk = {b: ([lambda b=b: k_chunk(b), lambda b=b: v_block(b)]
              + [lambda b=b, p=p: q_chunk(b, p) for p in range(4)])
          for b in range(1, 4)}

    items = []   # (qc, hp, fillers) — o_proj groups of finished blocks are
    # the only "storable" PE work, so they are pushed as late as dependency
    # rules allow to cover the exp-heavy tail items; qkv chunks cover the
    # early deficits and are spread to avoid rope pileups on the DVE FIFO.
    items += [(0, 0, qk[1][0:2]), (0, 1, [qk[1][2], qk[1][3]]),
              (0, 2, [qk[1][4]]), (0, 3, [qk[1][5]])]
    items += [(1, 0, qk[2][0:2]),
              (1, 1, qk[2][2:4]),
              (1, 2, [qk[2][4], qk[2][5], qk[3][0]]),
              (1, 3, [qk[3][1], qk[3][2]])]
    items += [(2, 0, [qk[3][3], qk[3][4]]),
              (2, 1, [qk[3][5], om(0, 0)]),
              (3, 0, [om(0, g) for g in range(1, 7)]),
              (2, 2, [om(0, g) for g in range(7, 13)]),
              (3, 1, [om(0, g) for g in range(13, 16)]
                     + [om(1, g) for g in range(0, 5)]),
              (2, 3, [om(1, g) for g in range(5, 11)]),
              (3, 2, [om(1, g) for g in range(11, 16)]
                     + [om(2, g) for g in range(0, 6)]),
              (3, 3, [om(2, g) for g in range(6, 16)])]

    q_chunk(0, 0)
    x_dmas([1, 2, 3])
    nc.sync.dma_start(out=wo_sb[:],
                      in_=wo[:].rearrange("(k p) d -> p k d", p=128))
    k_chunk(0)
    q_chunk(0, 1)
    v_block(0)
    q_chunk(0, 2)
    q_chunk(0, 3)
    for qc, hp, fillers in items:
        attn_pair(qc, hp, list(fillers))
        if hp == 1:
            o_xbars_f(qc, 0)
            o_xbars_f(qc, 2)
        elif hp == 3:
            o_xbars_f(qc, 1)
            o_xbars_f(qc, 3)
    for g in range(16):
        o_mm_group(3, g)
    ctx.close()


_NC_CACHE = None


def _get_nc():
    global _NC_CACHE
    if _NC_CACHE is None:
        _NC_CACHE = _build_nc()
    return _NC_CACHE


# interleaved head-dim order: row 2i = d_i, row 2i+1 = d_{i+32}
_PHI = np.empty(64, dtype=np.int64)
_PHI[0::2] = np.arange(32)
_PHI[1::2] = np.arange(32) + 32


def _rope_tables_T(pos):
    """Transposed rope tables in the interleaved row order, [128, SEQ]."""
    pos = np.asarray(pos, dtype=np.float32)
    inv = (1.0 / (np.float32(ROPE_THETA)
                  ** (np.arange(0, HEAD_DIM, 2, dtype=np.float32)
                      / np.float32(HEAD_DIM)))).astype(np.float32)  # [32]
    # row r (within 64): dim pair index i = r//2; angle = pos * inv[i]
    ang = inv[(np.arange(64) // 2)][:, None] * pos[None, :]   # [64, SEQ]
    c = np.cos(ang)
    s = np.sin(ang)
    sign = np.where(np.arange(64) % 2 == 0, -1.0, 1.0).astype(np.float32)
    ss = s * sign[:, None]
    cosT = np.concatenate([c, c], axis=0).astype(np.float32)   # [128, SEQ]
    ssT = np.concatenate([ss, ss], axis=0).astype(np.float32)
    return cosT, ssT


def _make_in_maps(input_ids, Wq, Wk, Wv, Wo, position_ids):
    x = np.asarray(input_ids, dtype=np.float32)
    Wq = np.asarray(Wq, dtype=np.float32)
    Wk = np.asarray(Wk, dtype=np.float32)
    Wv = np.asarray(Wv, dtype=np.float32)
    Wo = np.asarray(Wo, dtype=np.float32)
    pos = np.asarray(position_ids)

    maskt = np.triu(np.ones((128, 128), dtype=np.float32)).astype(BF16)

    in_maps = []
    for c in range(N_CORES):
        b, g = c // TP, c % TP
        xTc = np.ascontiguousarray(x[b].T).astype(BF16)
        # q pair-stacked stationaries: pair p = local heads (p, p+4),
        # columns phi-permuted within each head
        qcols = []
        for p in range(4):
            for hh in (p, p + 4):
                base = (g * QH + hh) * HEAD_DIM
                qcols.extend((base + _PHI).tolist())
        wq_t = Wq[:, qcols]                                    # [H, 512]
        # k stacked stationary: kv0 then kv1, phi-permuted
        kcols = []
        for j in range(KVH):
            base = (g * KVH + j) * HEAD_DIM
            kcols.extend((base + _PHI).tolist())
        wk_t = Wk[:, kcols]                                    # [H, 128]
        # v natural
        wv_n = Wv[:, g * KVH * HEAD_DIM:(g + 1) * KVH * HEAD_DIM]
        wall = np.concatenate([wq_t, wk_t, wv_n], axis=1).astype(BF16)
        wo_s = np.ascontiguousarray(
            Wo[g * F_O:(g + 1) * F_O, :]).astype(BF16)
        cosT, ssT = _rope_tables_T(pos[b])
        in_maps.append({
            "xT": np.ascontiguousarray(xTc),
            "wall": np.ascontiguousarray(wall),
            "wo": wo_s,
            "cosT": cosT,
            "ssT": ssT,
            "maskt": maskt,
        })
    return in_maps


def _run(in_maps, trace=False):
    nc = _get_nc()
    kwargs = {}
    if trace:
        _install_profile_hook()
        kwargs["trace"] = True
    return run_bass_kernel_spmd(nc, in_maps, core_ids=list(range(N_CORES)),
                                **kwargs)


def _install_profile_hook():
    """This image's antenv lacks axon_hooks; register the NTFF profile hook
    manually so trace=True yields hardware exec times."""
    if "antenv.axon_hooks" in sys.modules:
        return
    import antenv
    mod = types.ModuleType("antenv.axon_hooks")
    state = {"hook": None}
    mod.set_axon_ntff_profile_hook = lambda h: state.__setitem__("hook", h)
    mod.get_axon_ntff_profile_hook = lambda: state["hook"]
    sys.modules["antenv.axon_hooks"] = mod
    antenv.axon_hooks = mod
    try:
        from trn_agent_boot.trn_boot import _ntff_profile_via_ctypes
        mod.set_axon_ntff_profile_hook(
            _ntff_profile_via_ctypes("/opt/axon/libaxon_pjrt.so"))
    except Exception:
        pass


def kernel(input_ids, Wq, Wk, Wv, Wo, position_ids):
    in_maps = _make_in_maps(input_ids, Wq, Wk, Wv, Wo, position_ids)
    res = _run(in_maps, trace=bool(os.environ.get("KERNEL_TRACE")))
    if os.environ.get("KERNEL_TRACE"):
        print(f"HW exec time: {res.exec_time_ns} ns "
              f"(mean {res.mean_exec_time_ns})")
    out = np.zeros((BATCH, SEQ, HIDDEN), dtype=np.float32)
    for c in range(N_CORES):
        out[c // TP] += res.results[c]["out"]
    return out
